# revision 5
# baseline (speedup 1.0000x reference)
"""GQA attention (S=2048, D=4096, 32 Q heads / 8 KV heads, RoPE, full attn)
distributed over 8 Trainium2 NeuronCores.

Strategy (tensor-parallel by heads, local-wo + ReduceScatter):
  - core c owns Q heads 4c..4c+3 and KV head c (GQA groups align with cores).
  - projections as transposed GEMMs: QT/KT [chan, tok] directly usable by
    the scores matmul; V via VT + PE transposes; all big matmuls bf16
    (Fast Weight Load), f32 PSUM accumulate.
  - RoPE on the DVE only: with host-deinterleaved channels, r1 = u_lo - u_hi
    and r2 = v_lo + v_hi where u = src*cs1, v = src*cs2 (partition-split
    operands; PSUM src + SBUF table makes the mixed-base read legal).
  - scores transposed, ST = KT.T @ QT -> [k, q]; exp on ScalarE (bf16 out);
    softmax normalizer: bf16 DVE partial sums -> ones-matmul -> ln/exp on
    ScalarE -> f32r broadcast-matmul -> DVE scale.
  - output projection is LOCAL: out_partial[tok, 4096] = at_local.T @ woT
    (stationary = attention tile, moving = woT blocks), quantized bf16 and
    ReduceScattered per q-chunk; rank c's shard is exactly its out columns.
    No AllGather, no gather staging, and wo never waits on a collective.
  - wo(qc) matmuls are WOVEN into attn(qc+1)'s kt loop (4 per slot): the PE
    queue never idles while the scalar engine streams exps.
  - a full-sized dummy ReduceScatter during the projections absorbs the
    ncfw cold-start + RDH algorithm first-use cost (~60us -> ~25us).
  - consecutive matmuls that reuse the same bf16 stationary operand get
    ldweights=False patched post-schedule (skips the ~50ns serialized
    weight load; scores/PV pairs, wo jo-pairs, zps pairs).

Host side only reshapes/transposes/pads/casts and concatenates outputs
(final bf16 -> f32 upcast included).
"""
import sys

import numpy as np
import ml_dtypes

_BF16 = ml_dtypes.bfloat16

for _p in ("/root/.axon_site/_ro/trn_rl_repo", "/opt/trn_rl_repo"):
    if _p not in sys.path:
        sys.path.append(_p)

import concourse.bass as bass
import concourse.tile as tile
from concourse import mybir
from concourse.bass_utils import run_bass_kernel_spmd

N_CORES = 8
S = 2048
D = 4096
HD = 128
N_QH = 4          # Q heads per core
N_KT = S // 128   # 16 k-tiles
N_TC = S // 512   # 4 token chunks
N_KC = D // 128   # 32 contraction tiles
F32 = mybir.dt.float32
F32R = mybir.dt.float32r
BF16 = mybir.dt.bfloat16

_NC_CACHE = {}


def _split_multi_waits(nc):
    """This container's walrus accepts only ONE sync-wait per instruction
    encoding; hoist extra waits onto fresh single-wait NoOps placed before
    the instruction on the same engine."""
    n = 0
    for fn in nc.m.functions:
        for bb in fn.blocks:
            new_insts = []
            changed = False
            for ins in bb.instructions:
                si = ins.sync_info
                waits = list(si.on_wait) if si is not None else []
                if len(waits) > 1:
                    for w in waits[:-1]:
                        n += 1
                        nop = mybir.InstNoOp(name=f"WSPL-{n}", ins=[], outs=[])
                        nop.engine = ins.engine
                        nop.sync_info = mybir.SyncInfo(on_wait=[w], on_update=[])
                        new_insts.append(nop)
                    si.on_wait = waits[-1:]
                    changed = True
                new_insts.append(ins)
            if changed:
                bb.instructions = new_insts
    return n


def _dedup_ldweights(nc):
    """Consecutive matmuls (no other tensor-engine instruction between)
    with an identical bf16 stationary operand: the later ones reuse the
    already-loaded PE array weights (ldweights=False). f32/f32r stationary
    is excluded (non-self-loading f32r matmuls return zeros on HW)."""
    n = 0
    for fn in nc.m.functions:
        for bb in fn.blocks:
            prev = None  # (engine, weights_repr) of last tensor-engine inst
            for ins in bb.instructions:
                if isinstance(ins, mybir.InstMatmult):
                    if ins.is_transpose:
                        prev = None
                        continue
                    w = ins.ins[1]
                    wrep = str(w)
                    ok_dtype = "bfloat16" in wrep
                    if (
                        prev is not None
                        and wrep == prev
                        and ok_dtype
                        and ins.perf_mode is None
                    ):
                        ins.ldweights = False
                        n += 1
                    prev = wrep
                elif isinstance(ins, mybir.InstLdweights):
                    prev = None
                # other engines' instructions don't touch PE weights
    return n


def _build():
    nc = bass.Bass()

    xt = nc.dram_tensor("xt", [D, S], BF16, kind="ExternalInput")
    wqt = nc.dram_tensor("wqt", [D, 512], BF16, kind="ExternalInput")
    wkt = nc.dram_tensor("wkt", [D, HD], BF16, kind="ExternalInput")
    wvt = nc.dram_tensor("wvt", [D, HD], BF16, kind="ExternalInput")
    wot = nc.dram_tensor("wot", [128, N_QH, D], BF16, kind="ExternalInput")
    cs1 = nc.dram_tensor("cs1", [HD, S], F32, kind="ExternalInput")
    cs2 = nc.dram_tensor("cs2", [HD, S], F32, kind="ExternalInput")
    onesc = nc.dram_tensor("onesc", [HD, 1], BF16, kind="ExternalInput")
    onesr = nc.dram_tensor("onesr", [1, HD], F32R, kind="ExternalInput")
    ident = nc.dram_tensor("ident", [HD, HD], BF16, kind="ExternalInput")
    out_ext = nc.dram_tensor("out", [S, 512], BF16, kind="ExternalOutput")

    # per-chunk ReduceScatter buffers: rs_in[b, t, n] holds the partial
    # contribution to out columns b*512..(b+1)*512 -> rank b receives the
    # b-th contiguous shard of the sum = its own out columns.
    rs_in = [
        nc.dram_tensor(f"rsi{qc}", [N_CORES, 512, 512], BF16) for qc in range(N_TC)
    ]
    rs_out = [
        nc.dram_tensor(f"rso{qc}", [512, 512], BF16) for qc in range(N_TC)
    ]
    # full-sized dummy RS fired during the projections: absorbs the ncfw
    # cold-start AND the RDH-algorithm first-use cost so RS0 runs hot.
    warm_in = nc.dram_tensor("warmi", [N_CORES, 512, 512], BF16)
    warm_out = nc.dram_tensor("warmo", [512, 512], BF16)

    xt_r = xt.rearrange("(kc p) s -> kc p s", p=128)
    wqt_r = wqt.rearrange("(kc p) n -> kc p n", p=128)
    wkt_r = wkt.rearrange("(kc p) n -> kc p n", p=128)
    wvt_r = wvt.rearrange("(kc p) n -> kc p n", p=128)
    out_r = out_ext.rearrange("(qc ts p) n -> qc p ts n", qc=N_TC, p=128)

    with tile.TileContext(nc) as tc:
        with (
            tc.tile_pool(name="const", bufs=1) as constp,
            tc.tile_pool(name="persist", bufs=1) as persist,
        ):
            onesc_sb = constp.tile([HD, 1], BF16)
            onesr_sb = constp.tile([1, HD], F32R)

            qt_sb = persist.tile([128, N_QH, S], BF16)   # QT_rope
            kt_sb = persist.tile([128, S], BF16)         # KT_rope
            v_sb = persist.tile([128, N_KT, HD], BF16)   # V [tok-in-tile, kt, chan]

            # ---------------- phase 1: projections + rope ----------------
            with (
                tc.tile_pool(name="wq", bufs=1) as wqp,
                tc.tile_pool(name="csp", bufs=1) as csp,
                tc.tile_pool(name="xtp", bufs=3) as xtp,
                tc.tile_pool(name="uv", bufs=2) as uvp,
                tc.tile_pool(name="vt", bufs=2) as vtp,
                tc.tile_pool(name="p1q", bufs=1, space="PSUM") as p1q,
                tc.tile_pool(name="p1k", bufs=1, space="PSUM") as p1k,
                tc.tile_pool(name="p1r", bufs=1, space="PSUM") as p1r,
            ):
                wq_sb = wqp.tile([128, N_KC, 512], BF16)
                wk_sb = wqp.tile([128, N_KC, HD], BF16)
                wv_sb = wqp.tile([128, N_KC, HD], BF16)
                cs1_sb = csp.tile([HD, S], F32)
                cs2_sb = csp.tile([HD, S], F32)
                ident_sb = csp.tile([HD, HD], BF16)

                nc.gpsimd.collective_compute(
                    "ReduceScatter",
                    mybir.AluOpType.add,
                    replica_groups=[list(range(N_CORES))],
                    ins=[warm_in[:].opt()],
                    outs=[warm_out[:].opt()],
                )
                # first-needed first: wq kc 0..7, then wk/wv (kc 0 of the
                # K/V matmuls follows immediately), then the rest.
                nc.gpsimd.dma_start(
                    out=wq_sb[:, 0:8, :],
                    in_=wqt_r[0:8].rearrange("kc p n -> p kc n"),
                )
                nc.gpsimd.dma_start(out=wk_sb[:], in_=wkt_r[:].rearrange("kc p n -> p kc n"))
                nc.gpsimd.dma_start(out=wv_sb[:], in_=wvt_r[:].rearrange("kc p n -> p kc n"))
                for ch in (1, 2, 3):
                    nc.gpsimd.dma_start(
                        out=wq_sb[:, ch * 8:(ch + 1) * 8, :],
                        in_=wqt_r[ch * 8:(ch + 1) * 8].rearrange("kc p n -> p kc n"),
                    )
                nc.gpsimd.dma_start(out=cs1_sb[:], in_=cs1[:])
                nc.gpsimd.dma_start(out=cs2_sb[:], in_=cs2[:])
                nc.gpsimd.dma_start(out=onesc_sb[:], in_=onesc[:])
                nc.gpsimd.dma_start(out=onesr_sb[:], in_=onesr[:])
                nc.gpsimd.dma_start(out=ident_sb[:], in_=ident[:])

                for tcb in range(N_TC):
                    t0 = tcb * 512
                    scope = nc.named_scope(f"proj{tcb}"); scope.__enter__()
                    qps = [
                        p1q.tile([128, 512], F32, name=f"qps{tcb}_{h}", tag=f"qps{h}")
                        for h in range(N_QH)
                    ]
                    kps = p1k.tile([128, 512], F32, name=f"kps{tcb}", tag="kps")
                    vtps = p1k.tile([128, 512], F32, name=f"vtps{tcb}", tag="vtps")
                    xt_g = None
                    for kc in range(N_KC):
                        if kc % 8 == 0:
                            xt_g = xtp.tile([128, 8, 512], BF16, name=f"xt{tcb}_{kc}", tag="xt")
                            nc.sync.dma_start(
                                out=xt_g[:],
                                in_=xt_r[kc:kc + 8, :, t0:t0 + 512].rearrange("g p n -> p g n"),
                            )
                        xt_t = xt_g[:, kc % 8, :]
                        st, sp = kc == 0, kc == N_KC - 1
                        for h in range(N_QH):
                            nc.tensor.matmul(
                                qps[h][:], wq_sb[:, kc, h * 128:(h + 1) * 128],
                                xt_t, start=st, stop=sp,
                            )
                        nc.tensor.matmul(kps[:], wk_sb[:, kc, :], xt_t, start=st, stop=sp)
                        nc.tensor.matmul(vtps[:], wv_sb[:, kc, :], xt_t, start=st, stop=sp)

                    # rope on DVE: K first (attention depends on full KT).
                    # u = src*cs1 = [v1*c ; v2*s], v = src*cs2 = [v1*s ; v2*c]
                    # staged as [64, 2, 512] (lo/hi side by side in the free
                    # dim; PSUM src keeps the mixed-base mul legal), then
                    # r = [u_lo - u_hi ; v_lo + v_hi] with same-base operands.
                    for h in [N_QH] + list(range(N_QH)):
                        src = kps if h == N_QH else qps[h]
                        u_t = uvp.tile([64, 2, 512], BF16, name=f"u{tcb}_{h}", tag="u")
                        v_t = uvp.tile([64, 2, 512], BF16, name=f"v{tcb}_{h}", tag="v")
                        nc.vector.tensor_mul(u_t[:, 0, :], src[0:64, :], cs1_sb[0:64, t0:t0 + 512])
                        nc.vector.tensor_mul(u_t[:, 1, :], src[64:128, :], cs1_sb[64:128, t0:t0 + 512])
                        nc.vector.tensor_mul(v_t[:, 0, :], src[0:64, :], cs2_sb[0:64, t0:t0 + 512])
                        nc.vector.tensor_mul(v_t[:, 1, :], src[64:128, :], cs2_sb[64:128, t0:t0 + 512])
                        if h == N_QH:
                            d_top = kt_sb[0:64, t0:t0 + 512]
                            d_bot = kt_sb[64:128, t0:t0 + 512]
                        else:
                            d_top = qt_sb[0:64, h, t0:t0 + 512]
                            d_bot = qt_sb[64:128, h, t0:t0 + 512]
                        nc.vector.tensor_sub(d_top, u_t[:, 0, :], u_t[:, 1, :])
                        nc.vector.tensor_add(d_bot, v_t[:, 0, :], v_t[:, 1, :])

                    # V for this token chunk: VT -> PE transpose -> V
                    vt_sb = vtp.tile([128, 512], BF16, name=f"vts{tcb}", tag="vts")
                    nc.vector.tensor_copy(vt_sb[:], vtps[:])
                    vtr = p1r.tile([128, 4, 128], BF16, name=f"vtr{tcb}", tag="vtr")
                    for j in range(4):
                        nc.tensor.transpose(
                            vtr[:, j, :], vt_sb[:, j * 128:(j + 1) * 128],
                            ident_sb[:],
                        )
                    nc.vector.tensor_copy(v_sb[:, tcb * 4:(tcb + 1) * 4, :], vtr[:])
                    scope.__exit__(None, None, None)

            # -- phase 2: attention, with prev chunk's local wo GEMM woven in --
            with (
                tc.tile_pool(name="wo", bufs=1) as wop,
                tc.tile_pool(name="ep", bufs=6) as ep,
                tc.tile_pool(name="zp", bufs=1) as zp,
                tc.tile_pool(name="np_", bufs=2) as np_,
                tc.tile_pool(name="atp", bufs=2) as atp,
                tc.tile_pool(name="rsev", bufs=3) as rsev,
                tc.tile_pool(name="outp", bufs=2) as outp,
                tc.tile_pool(name="p2s", bufs=2, space="PSUM") as p2s,
                tc.tile_pool(name="p2pv", bufs=1, space="PSUM") as p2pv,
                tc.tile_pool(name="p2wo", bufs=1, space="PSUM") as p2wo,
            ):
                wo_sb = wop.tile([128, N_QH, D], BF16)
                nc.gpsimd.dma_start(out=wo_sb[:], in_=wot[:])

                at_t = {}      # (qc, h) -> normalized attention tile [128, 512]
                wo_wps = {}

                def wo_mm(qc, m):
                    # one wo matmul: m = ts*32 + q4*8 + ct*2 + jo
                    # out_partial[ts-tile, ob*512:+512] += at[ct][:,ts].T @ woT[ct, ob]
                    ts, q4 = m // 32, (m % 32) // 8
                    r = m % 8
                    ct, jo = r // 2, r % 2
                    if r == 0:
                        wo_wps[qc] = p2wo.tile(
                            [128, 2, 512], F32, name=f"wps{qc}_{m}", tag="wo"
                        )
                    wps = wo_wps[qc]
                    ob = q4 * 2 + jo
                    nc.tensor.matmul(
                        wps[:, jo, :],
                        at_t[(qc, ct)][:, ts * 128:(ts + 1) * 128],
                        wo_sb[:, ct, ob * 512:(ob + 1) * 512],
                        start=(ct == 0), stop=(ct == 3),
                    )
                    if r == 7:
                        ev = rsev.tile(
                            [128, 2, 512], BF16, name=f"wev{qc}_{m}", tag="wev"
                        )
                        nc.vector.tensor_copy(ev[:], wps[:])
                        for j2 in range(2):
                            nc.sync.dma_start(
                                out=rs_in[qc][q4 * 2 + j2, ts * 128:(ts + 1) * 128, :],
                                in_=ev[:, j2, :],
                            )
                    if m == 127:
                        sc2 = nc.named_scope(f"rs{qc}"); sc2.__enter__()
                        nc.gpsimd.collective_compute(
                            "ReduceScatter",
                            mybir.AluOpType.add,
                            replica_groups=[list(range(N_CORES))],
                            ins=[rs_in[qc][:].opt()],
                            outs=[rs_out[qc][:].opt()],
                        )
                        sc2.__exit__(None, None, None)
                        ob_t = outp.tile(
                            [128, N_TC, 512], BF16, name=f"outb{qc}", tag="outb"
                        )
                        nc.sync.dma_start(
                            out=ob_t[:],
                            in_=rs_out[qc].rearrange("(ts p) n -> p ts n", p=128),
                        )
                        nc.sync.dma_start(out=out_r[qc], in_=ob_t[:])

                def attn_chunk(qc, weave=None):
                    # weave: chunk index whose wo matmuls (128) are spread
                    # over this chunk's 32 kt slots, 4 per slot.
                    q0 = qc * 512
                    scope = nc.named_scope(f"attn{qc}"); scope.__enter__()
                    wv_n = [0]

                    def weave_step(upto):
                        if weave is None:
                            return
                        while wv_n[0] < min(upto, 128):
                            wo_mm(weave, wv_n[0])
                            wv_n[0] += 1

                    for hp in range(2):
                        hs = [2 * hp, 2 * hp + 1]
                        pvs = {
                            h: p2pv.tile([128, 512], F32, name=f"pv{qc}_{h}", tag=f"pv{h % 2}")
                            for h in hs
                        }
                        zparts = {
                            h: zp.tile([128, 512], BF16, name=f"zpt{qc}_{h}", tag=f"zpart{h % 2}")
                            for h in hs
                        }
                        for kt in range(N_KT):
                            k0 = kt * 128
                            stp = p2s.tile([128, 2, 512], F32, name=f"st{qc}_{hp}_{kt}", tag="st")
                            for j, h in enumerate(hs):
                                nc.tensor.matmul(
                                    stp[:, j, :], kt_sb[:, k0:k0 + 128],
                                    qt_sb[:, h, q0:q0 + 512], start=True, stop=True,
                                )
                            e_t = ep.tile([128, 2, 512], BF16, name=f"e{qc}_{kt}_{hp}", tag="e")
                            nc.scalar.activation(
                                out=e_t[:], in_=stp[:],
                                func=mybir.ActivationFunctionType.Exp,
                            )
                            for j, h in enumerate(hs):
                                if kt == 0:
                                    nc.vector.tensor_copy(zparts[h][:], e_t[:, j, :])
                                else:
                                    nc.vector.tensor_add(zparts[h][:], zparts[h][:], e_t[:, j, :])
                            for j, h in enumerate(hs):
                                nc.tensor.matmul(
                                    pvs[h][:], v_sb[:, kt, :], e_t[:, j, :],
                                    start=(kt == 0), stop=(kt == N_KT - 1),
                                )
                            weave_step((hp * N_KT + kt + 1) * 4)
                        zpss = {}
                        for h in hs:
                            zpss[h] = p2s.tile([1, 512], F32, name=f"zps{qc}_{h}", tag="st")
                            nc.tensor.matmul(zpss[h][:], onesc_sb[:], zparts[h][:], start=True, stop=True)
                        invzs = {}
                        for h in hs:
                            lnz = zp.tile([1, 512], F32, name=f"lnz{qc}_{h}", tag=f"lnz{h % 2}")
                            nc.scalar.activation(
                                out=lnz[:], in_=zpss[h][:],
                                func=mybir.ActivationFunctionType.Ln,
                            )
                            invzs[h] = zp.tile([1, 512], F32R, name=f"izr{qc}_{h}", tag=f"invz{h % 2}")
                            nc.scalar.activation(
                                out=invzs[h][:], in_=lnz[:],
                                func=mybir.ActivationFunctionType.Exp, scale=-1.0,
                            )
                        for h in hs:
                            bcps = p2s.tile([128, 512], F32, name=f"bc{qc}_{h}", tag="st")
                            nc.tensor.matmul(bcps[:], onesr_sb[:], invzs[h][:], start=True, stop=True)
                            bc_sb = np_.tile([128, 512], F32, name=f"bcs{qc}_{h}", tag="bcs")
                            nc.vector.tensor_copy(bc_sb[:], bcps[:])
                            a_t = atp.tile([128, 512], BF16, name=f"at{qc}_{h}", tag=f"at{h}")
                            nc.vector.tensor_mul(a_t[:], pvs[h][:], bc_sb[:])
                            at_t[(qc, h)] = a_t
                    weave_step(128)
                    scope.__exit__(None, None, None)

                attn_chunk(0)
                attn_chunk(1, weave=0)
                attn_chunk(2, weave=1)
                attn_chunk(3, weave=2)
                scope = nc.named_scope("wo3"); scope.__enter__()
                for m in range(128):
                    wo_mm(3, m)
                scope.__exit__(None, None, None)

    _dedup_ldweights(nc)
    _split_multi_waits(nc)
    return nc


def _host_prep(x, cos, sin, wq, wk, wv, wo):
    scale = np.float32(HD ** -0.5)
    perm = np.concatenate([np.arange(0, HD, 2), np.arange(1, HD, 2)])

    xT = np.ascontiguousarray(x.T.astype(_BF16))
    cosT = np.ascontiguousarray(cos.T)
    sinT = np.ascontiguousarray(sin.T)
    cs1 = np.concatenate([cosT, sinT], axis=0)
    cs2 = np.concatenate([sinT, cosT], axis=0)

    shared = {
        "xt": xT,
        "cs1": cs1,
        "cs2": cs2,
        "onesc": np.ones((HD, 1), np.float32).astype(_BF16),
        "onesr": np.ones((1, HD), np.float32),
        "ident": np.eye(HD, dtype=np.float32).astype(_BF16),
    }
    in_maps = []
    for c in range(N_CORES):
        wq_c = wq[c * 512:(c + 1) * 512].reshape(N_QH, HD, D)[:, perm, :]
        wq_c = (wq_c.reshape(512, D) * scale)
        wk_c = wk[c * HD:(c + 1) * HD][perm, :]
        wv_c = wv[c * HD:(c + 1) * HD]
        # woT for the local-wo GEMM: [p, ct, n] with chan = c*512 + ct*128 + p
        wo_c = np.ascontiguousarray(
            wo[:, c * 512:(c + 1) * 512].T.reshape(N_QH, 128, D).transpose(1, 0, 2)
        )
        m = dict(shared)
        m["wqt"] = np.ascontiguousarray(wq_c.T).astype(_BF16)
        m["wkt"] = np.ascontiguousarray(wk_c.T).astype(_BF16)
        m["wvt"] = np.ascontiguousarray(wv_c.T).astype(_BF16)
        m["wot"] = wo_c.astype(_BF16)
        in_maps.append(m)
    return in_maps


def kernel(x, cos, sin, wq, wk, wv, wo, _trace=False):
    x = np.asarray(x, np.float32)
    cos = np.asarray(cos, np.float32)
    sin = np.asarray(sin, np.float32)
    wq = np.asarray(wq, np.float32)
    wk = np.asarray(wk, np.float32)
    wv = np.asarray(wv, np.float32)
    wo = np.asarray(wo, np.float32)

    in_maps = _host_prep(x, cos, sin, wq, wk, wv, wo)
    if "nc" not in _NC_CACHE:
        _NC_CACHE["nc"] = _build()
    nc = _NC_CACHE["nc"]
    res = run_bass_kernel_spmd(
        nc, in_maps, core_ids=list(range(N_CORES)), trace=_trace
    )
    out = np.concatenate(
        [np.asarray(res.results[c]["out"]) for c in range(N_CORES)], axis=1
    )
    out = np.ascontiguousarray(out.astype(np.float32))
    if _trace:
        kernel._last_exec_time_ns = res.exec_time_ns
        kernel._last_result = res
    return out


# revision 14
# speedup vs baseline: 1.1344x; 1.1344x over previous
"""GQA attention (S=2048, D=4096, 32 Q heads / 8 KV heads, RoPE, full attn)
distributed over 8 Trainium2 NeuronCores.

Strategy (tensor-parallel by heads, local-wo + ReduceScatter):
  - core c owns Q heads 4c..4c+3 and KV head c (GQA groups align with cores).
  - projections as transposed GEMMs: QT/KT [chan, tok] directly usable by
    the scores matmul; V via VT + PE transposes; all big matmuls bf16
    (Fast Weight Load), f32 PSUM accumulate.
  - RoPE on the DVE only: with host-deinterleaved channels, r1 = u_lo - u_hi
    and r2 = v_lo + v_hi where u = src*cs1, v = src*cs2 (partition-split
    operands; PSUM src + SBUF table makes the mixed-base read legal).
  - scores transposed, ST = KT.T @ QT -> [k, q]; exp on ScalarE (bf16 out);
    softmax normalizer: bf16 DVE partial sums -> ones-matmul -> ln/exp on
    ScalarE -> f32r broadcast-matmul -> DVE scale.
  - output projection is LOCAL: out_partial[tok, 4096] = at_local.T @ woT
    (stationary = attention tile, moving = woT blocks), quantized bf16 and
    ReduceScattered per q-chunk; rank c's shard is exactly its out columns.
    No AllGather, no gather staging, and wo never waits on a collective.
  - wo(qc) matmuls are WOVEN into attn(qc+1)'s kt loop (4 per slot): the PE
    queue never idles while the scalar engine streams exps.
  - a full-sized dummy ReduceScatter during the projections absorbs the
    ncfw cold-start + RDH algorithm first-use cost (~60us -> ~25us).
  - consecutive matmuls that reuse the same bf16 stationary operand get
    ldweights=False patched post-schedule (skips the ~50ns serialized
    weight load; scores/PV pairs, wo jo-pairs, zps pairs).

Host side only reshapes/transposes/pads/casts and concatenates outputs
(final bf16 -> f32 upcast included).
"""
import sys

import numpy as np
import ml_dtypes

_BF16 = ml_dtypes.bfloat16

for _p in ("/root/.axon_site/_ro/trn_rl_repo", "/opt/trn_rl_repo"):
    if _p not in sys.path:
        sys.path.append(_p)

import concourse.bass as bass
import concourse.tile as tile
from concourse import mybir
from concourse.bass_utils import run_bass_kernel_spmd

N_CORES = 8
S = 2048
D = 4096
HD = 128
N_QH = 4          # Q heads per core
N_KT = S // 128   # 16 k-tiles
N_TC = S // 512   # 4 token chunks
N_KC = D // 128   # 32 contraction tiles
F32 = mybir.dt.float32
F32R = mybir.dt.float32r
BF16 = mybir.dt.bfloat16

_NC_CACHE = {}


def _split_multi_waits(nc):
    """This container's walrus accepts only ONE sync-wait per instruction
    encoding; hoist extra waits onto fresh single-wait NoOps placed before
    the instruction on the same engine."""
    n = 0
    for fn in nc.m.functions:
        for bb in fn.blocks:
            new_insts = []
            changed = False
            for ins in bb.instructions:
                si = ins.sync_info
                waits = list(si.on_wait) if si is not None else []
                if len(waits) > 1:
                    for w in waits[:-1]:
                        n += 1
                        nop = mybir.InstNoOp(name=f"WSPL-{n}", ins=[], outs=[])
                        nop.engine = ins.engine
                        nop.sync_info = mybir.SyncInfo(on_wait=[w], on_update=[])
                        new_insts.append(nop)
                    si.on_wait = waits[-1:]
                    changed = True
                new_insts.append(ins)
            if changed:
                bb.instructions = new_insts
    return n


def _dedup_ldweights(nc):
    """Consecutive matmuls (no other tensor-engine instruction between)
    with an identical bf16 stationary operand: the later ones reuse the
    already-loaded PE array weights (ldweights=False). f32/f32r stationary
    is excluded (non-self-loading f32r matmuls return zeros on HW)."""
    n = 0
    for fn in nc.m.functions:
        for bb in fn.blocks:
            prev = None  # (engine, weights_repr) of last tensor-engine inst
            for ins in bb.instructions:
                if isinstance(ins, mybir.InstMatmult):
                    if ins.is_transpose:
                        prev = None
                        continue
                    w = ins.ins[1]
                    wrep = str(w)
                    ok_dtype = "bfloat16" in wrep
                    if (
                        prev is not None
                        and wrep == prev
                        and ok_dtype
                        and ins.perf_mode is None
                    ):
                        ins.ldweights = False
                        n += 1
                    prev = wrep
                elif isinstance(ins, mybir.InstLdweights):
                    prev = None
                # other engines' instructions don't touch PE weights
    return n


def _build():
    nc = bass.Bass()

    xt = nc.dram_tensor("xt", [D, S], BF16, kind="ExternalInput")
    wqt = nc.dram_tensor("wqt", [D, 512], BF16, kind="ExternalInput")
    wkt = nc.dram_tensor("wkt", [D, HD], BF16, kind="ExternalInput")
    wvt = nc.dram_tensor("wvt", [D, HD], BF16, kind="ExternalInput")
    wot = nc.dram_tensor("wot", [128, N_QH, D], BF16, kind="ExternalInput")
    cs1 = nc.dram_tensor("cs1", [HD, S], F32, kind="ExternalInput")
    cs2 = nc.dram_tensor("cs2", [HD, S], F32, kind="ExternalInput")
    onesc = nc.dram_tensor("onesc", [HD, 1], BF16, kind="ExternalInput")
    onesr = nc.dram_tensor("onesr", [1, HD], F32R, kind="ExternalInput")
    ident = nc.dram_tensor("ident", [HD, HD], BF16, kind="ExternalInput")
    out_ext = nc.dram_tensor("out", [S, 512], BF16, kind="ExternalOutput")

    # per-chunk ReduceScatter buffers: rs_in[b, t, n] holds the partial
    # contribution to out columns b*512..(b+1)*512 -> rank b receives the
    # b-th contiguous shard of the sum = its own out columns. The last
    # chunk is split into two token halves so the first RS overlaps the
    # second half of wo3 (shorter serial tail).
    rs_in = [
        nc.dram_tensor(f"rsi{qc}", [N_CORES, 512, 512], BF16) for qc in range(3)
    ] + [
        nc.dram_tensor(f"rsi3{h}", [N_CORES, 256, 512], BF16) for h in range(2)
    ]
    rs_out = [
        nc.dram_tensor(f"rso{qc}", [512, 512], BF16) for qc in range(3)
    ] + [
        nc.dram_tensor(f"rso3{h}", [256, 512], BF16) for h in range(2)
    ]
    # full-sized dummy RS fired during the projections: absorbs the ncfw
    # cold-start AND the RDH-algorithm first-use cost so RS0 runs hot.
    warm_in = nc.dram_tensor("warmi", [N_CORES, 512, 512], BF16)
    warm_out = nc.dram_tensor("warmo", [512, 512], BF16)

    xt_r = xt.rearrange("(kc p) s -> kc p s", p=128)
    wqt_r = wqt.rearrange("(kc p) n -> kc p n", p=128)
    wkt_r = wkt.rearrange("(kc p) n -> kc p n", p=128)
    wvt_r = wvt.rearrange("(kc p) n -> kc p n", p=128)


    with tile.TileContext(nc) as tc:
        with (
            tc.tile_pool(name="const", bufs=1) as constp,
            tc.tile_pool(name="persist", bufs=1) as persist,
        ):
            onesc_sb = constp.tile([HD, 1], BF16)
            onesr_sb = constp.tile([1, HD], F32R)

            qt_sb = persist.tile([128, N_QH, S], BF16)   # QT_rope
            kt_sb = persist.tile([128, S], BF16)         # KT_rope
            v_sb = persist.tile([128, N_KT, HD], BF16)   # V [tok-in-tile, kt, chan]

            # ---------------- phase 1: projections + rope ----------------
            with (
                tc.tile_pool(name="wq", bufs=1) as wqp,
                tc.tile_pool(name="csp", bufs=1) as csp,
                tc.tile_pool(name="xtp", bufs=3) as xtp,
                tc.tile_pool(name="uv", bufs=2) as uvp,
                tc.tile_pool(name="vt", bufs=2) as vtp,
                tc.tile_pool(name="p1q", bufs=1, space="PSUM") as p1q,
                tc.tile_pool(name="p1k", bufs=1, space="PSUM") as p1k,
                tc.tile_pool(name="p1r", bufs=1, space="PSUM") as p1r,
            ):
                wq_sb = wqp.tile([128, N_KC, 512], BF16)
                wk_sb = wqp.tile([128, N_KC, HD], BF16)
                wv_sb = wqp.tile([128, N_KC, HD], BF16)
                cs1_sb = csp.tile([HD, S], F32)
                cs2_sb = csp.tile([HD, S], F32)
                ident_sb = csp.tile([HD, HD], BF16)

                nc.gpsimd.collective_compute(
                    "ReduceScatter",
                    mybir.AluOpType.add,
                    replica_groups=[list(range(N_CORES))],
                    ins=[warm_in[:].opt()],
                    outs=[warm_out[:].opt()],
                )
                # weights in 8-kc groups, interleaved wq/wk/wv so the kc
                # loop's consumption never outruns the DMA supply.
                for ch in range(4):
                    nc.gpsimd.dma_start(
                        out=wq_sb[:, ch * 8:(ch + 1) * 8, :],
                        in_=wqt_r[ch * 8:(ch + 1) * 8].rearrange("kc p n -> p kc n"),
                    )
                    nc.gpsimd.dma_start(
                        out=wk_sb[:, ch * 8:(ch + 1) * 8, :],
                        in_=wkt_r[ch * 8:(ch + 1) * 8].rearrange("kc p n -> p kc n"),
                    )
                    nc.gpsimd.dma_start(
                        out=wv_sb[:, ch * 8:(ch + 1) * 8, :],
                        in_=wvt_r[ch * 8:(ch + 1) * 8].rearrange("kc p n -> p kc n"),
                    )
                nc.gpsimd.dma_start(out=cs1_sb[:], in_=cs1[:])
                nc.gpsimd.dma_start(out=cs2_sb[:], in_=cs2[:])
                nc.gpsimd.dma_start(out=onesc_sb[:], in_=onesc[:])
                nc.gpsimd.dma_start(out=onesr_sb[:], in_=onesr[:])
                nc.gpsimd.dma_start(out=ident_sb[:], in_=ident[:])

                for tcb in range(N_TC):
                    t0 = tcb * 512
                    scope = nc.named_scope(f"proj{tcb}"); scope.__enter__()
                    qps = [
                        p1q.tile([128, 512], F32, name=f"qps{tcb}_{h}", tag=f"qps{h}")
                        for h in range(N_QH)
                    ]
                    kps = p1k.tile([128, 512], F32, name=f"kps{tcb}", tag="kps")
                    vtps = p1k.tile([128, 512], F32, name=f"vtps{tcb}", tag="vtps")
                    xt_g = None
                    for kc in range(N_KC):
                        if kc % 8 == 0:
                            xt_g = xtp.tile([128, 8, 512], BF16, name=f"xt{tcb}_{kc}", tag="xt")
                            nc.sync.dma_start(
                                out=xt_g[:],
                                in_=xt_r[kc:kc + 8, :, t0:t0 + 512].rearrange("g p n -> p g n"),
                            )
                        xt_t = xt_g[:, kc % 8, :]
                        st, sp = kc == 0, kc == N_KC - 1
                        for h in range(N_QH):
                            nc.tensor.matmul(
                                qps[h][:], wq_sb[:, kc, h * 128:(h + 1) * 128],
                                xt_t, start=st, stop=sp,
                            )
                        nc.tensor.matmul(kps[:], wk_sb[:, kc, :], xt_t, start=st, stop=sp)
                        nc.tensor.matmul(vtps[:], wv_sb[:, kc, :], xt_t, start=st, stop=sp)

                    # rope on DVE: K first (attention depends on full KT).
                    # With deinterleaved chans, cs1=[cos;-sin], cs2=[sin;cos]:
                    #   a = [v1*c ; v1*s] (both from src_lo),
                    #   b = [-v2*s ; v2*c] (both from src_hi),  r = a + b.
                    # Every mul keeps PSUM-in base == SBUF-in base (legal);
                    # only the out base differs. The add is one full-lane op.
                    for h in [N_QH] + list(range(N_QH)):
                        src = kps if h == N_QH else qps[h]
                        a_t = uvp.tile([128, 512], BF16, name=f"u{tcb}_{h}", tag="u")
                        b_t = uvp.tile([128, 512], BF16, name=f"v{tcb}_{h}", tag="v")
                        nc.vector.tensor_mul(a_t[0:64, :], src[0:64, :], cs1_sb[0:64, t0:t0 + 512])
                        nc.vector.tensor_mul(a_t[64:128, :], src[0:64, :], cs2_sb[0:64, t0:t0 + 512])
                        nc.vector.tensor_mul(b_t[0:64, :], src[64:128, :], cs1_sb[64:128, t0:t0 + 512])
                        nc.vector.tensor_mul(b_t[64:128, :], src[64:128, :], cs2_sb[64:128, t0:t0 + 512])
                        if h == N_QH:
                            dst = kt_sb[:, t0:t0 + 512]
                        else:
                            dst = qt_sb[:, h, t0:t0 + 512]
                        nc.vector.tensor_add(dst, a_t[:], b_t[:])

                    # V for this token chunk: VT -> PE transpose -> V
                    vt_sb = vtp.tile([128, 512], BF16, name=f"vts{tcb}", tag="vts")
                    nc.vector.tensor_copy(vt_sb[:], vtps[:])
                    vtr = p1r.tile([128, 4, 128], BF16, name=f"vtr{tcb}", tag="vtr")
                    for j in range(4):
                        nc.tensor.transpose(
                            vtr[:, j, :], vt_sb[:, j * 128:(j + 1) * 128],
                            ident_sb[:],
                        )
                    nc.vector.tensor_copy(v_sb[:, tcb * 4:(tcb + 1) * 4, :], vtr[:])
                    scope.__exit__(None, None, None)

            # -- phase 2: attention, with prev chunk's local wo GEMM woven in --
            with (
                tc.tile_pool(name="wo", bufs=1) as wop,
                tc.tile_pool(name="ep", bufs=6) as ep,
                tc.tile_pool(name="zp", bufs=1) as zp,
                tc.tile_pool(name="np_", bufs=2) as np_,
                tc.tile_pool(name="atp", bufs=2) as atp,
                tc.tile_pool(name="rsev", bufs=3) as rsev,
                tc.tile_pool(name="p2s", bufs=2, space="PSUM") as p2s,
                tc.tile_pool(name="p2pv", bufs=1, space="PSUM") as p2pv,
                tc.tile_pool(name="p2wo", bufs=1, space="PSUM") as p2wo,
            ):
                wo_sb = wop.tile([128, N_QH, D], BF16)
                nc.gpsimd.dma_start(out=wo_sb[:], in_=wot[:])

                at_t = {}      # (qc, h) -> normalized attention tile [128, 512]
                wo_wps = {}

                def fire_rs(ri, tok0, ntok):
                    # rs_in[ri] is complete: reduce-scatter it and copy this
                    # rank's shard to the output rows. Everything here lives
                    # on the gpsimd queue: it waits for the collective, but
                    # nothing latency-critical queues behind it there (the
                    # sync queue must stay free for the next chunk's rs_in
                    # writes or the whole weave wedges on buffer recycling).
                    sc2 = nc.named_scope(f"rs{ri}"); sc2.__enter__()
                    nc.gpsimd.collective_compute(
                        "ReduceScatter",
                        mybir.AluOpType.add,
                        replica_groups=[list(range(N_CORES))],
                        ins=[rs_in[ri][:].opt()],
                        outs=[rs_out[ri][:].opt()],
                    )
                    nc.gpsimd.dma_start(
                        out=out_ext[tok0:tok0 + ntok, :], in_=rs_out[ri][:]
                    )
                    sc2.__exit__(None, None, None)

                def wo_mm(qc, m):
                    # one wo matmul: m = ts*32 + q4*8 + ct*2 + jo
                    # out_partial[ts-tile, ob*512:+512] += at[ct][:,ts].T @ woT[ct, ob]
                    ts, q4 = m // 32, (m % 32) // 8
                    r = m % 8
                    ct, jo = r // 2, r % 2
                    if r == 0:
                        wo_wps[qc] = p2wo.tile(
                            [128, 2, 512], F32, name=f"wps{qc}_{m}", tag="wo"
                        )
                    wps = wo_wps[qc]
                    ob = q4 * 2 + jo
                    nc.tensor.matmul(
                        wps[:, jo, :],
                        at_t[(qc, ct)][:, ts * 128:(ts + 1) * 128],
                        wo_sb[:, ct, ob * 512:(ob + 1) * 512],
                        start=(ct == 0), stop=(ct == 3),
                    )
                    if r == 7:
                        ev = rsev.tile(
                            [128, 2, 512], BF16, name=f"wev{qc}_{m}", tag="wev"
                        )
                        nc.vector.tensor_copy(ev[:], wps[:])
                        if qc < 3:
                            ri, row = qc, ts * 128
                        else:
                            ri, row = 3 + ts // 2, (ts % 2) * 128
                        for j2 in range(2):
                            nc.sync.dma_start(
                                out=rs_in[ri][q4 * 2 + j2, row:row + 128, :],
                                in_=ev[:, j2, :],
                            )
                    if qc < 3:
                        if m == 127:
                            fire_rs(qc, qc * 512, 512)
                    else:
                        if m == 63:
                            fire_rs(3, 3 * 512, 256)
                        elif m == 127:
                            fire_rs(4, 3 * 512 + 256, 256)

                def attn_chunk(qc, weave=None):
                    # weave: chunk index whose wo matmuls (128) are spread
                    # over this chunk's 32 kt slots, 4 per slot.
                    q0 = qc * 512
                    scope = nc.named_scope(f"attn{qc}"); scope.__enter__()
                    wv_n = [0]

                    def weave_step(upto):
                        if weave is None:
                            return
                        while wv_n[0] < min(upto, 128):
                            wo_mm(weave, wv_n[0])
                            wv_n[0] += 1

                    for hp in range(2):
                        hs = [2 * hp, 2 * hp + 1]
                        pvs = {
                            h: p2pv.tile([128, 512], F32, name=f"pv{qc}_{h}", tag=f"pv{h % 2}")
                            for h in hs
                        }
                        zparts = {
                            h: zp.tile([128, 512], BF16, name=f"zpt{qc}_{h}", tag=f"zpart{h % 2}")
                            for h in hs
                        }
                        for kt in range(N_KT):
                            k0 = kt * 128
                            stp = p2s.tile([128, 2, 512], F32, name=f"st{qc}_{hp}_{kt}", tag="st")
                            for j, h in enumerate(hs):
                                nc.tensor.matmul(
                                    stp[:, j, :], kt_sb[:, k0:k0 + 128],
                                    qt_sb[:, h, q0:q0 + 512], start=True, stop=True,
                                )
                            e_t = ep.tile([128, 2, 512], BF16, name=f"e{qc}_{kt}_{hp}", tag="e")
                            nc.scalar.activation(
                                out=e_t[:], in_=stp[:],
                                func=mybir.ActivationFunctionType.Exp,
                            )
                            for j, h in enumerate(hs):
                                if kt == 0:
                                    nc.vector.tensor_copy(zparts[h][:], e_t[:, j, :])
                                else:
                                    nc.vector.tensor_add(zparts[h][:], zparts[h][:], e_t[:, j, :])
                            for j, h in enumerate(hs):
                                nc.tensor.matmul(
                                    pvs[h][:], v_sb[:, kt, :], e_t[:, j, :],
                                    start=(kt == 0), stop=(kt == N_KT - 1),
                                )
                            weave_step((hp * N_KT + kt + 1) * 4)
                        zpss = {}
                        for h in hs:
                            zpss[h] = p2s.tile([1, 512], F32, name=f"zps{qc}_{h}", tag="st")
                            nc.tensor.matmul(zpss[h][:], onesc_sb[:], zparts[h][:], start=True, stop=True)
                        invzs = {}
                        for h in hs:
                            lnz = zp.tile([1, 512], F32, name=f"lnz{qc}_{h}", tag=f"lnz{h % 2}")
                            nc.scalar.activation(
                                out=lnz[:], in_=zpss[h][:],
                                func=mybir.ActivationFunctionType.Ln,
                            )
                            invzs[h] = zp.tile([1, 512], F32R, name=f"izr{qc}_{h}", tag=f"invz{h % 2}")
                            nc.scalar.activation(
                                out=invzs[h][:], in_=lnz[:],
                                func=mybir.ActivationFunctionType.Exp, scale=-1.0,
                            )
                        for h in hs:
                            bcps = p2s.tile([128, 512], F32, name=f"bc{qc}_{h}", tag="st")
                            nc.tensor.matmul(bcps[:], onesr_sb[:], invzs[h][:], start=True, stop=True)
                            bc_sb = np_.tile([128, 512], F32, name=f"bcs{qc}_{h}", tag="bcs")
                            nc.vector.tensor_copy(bc_sb[:], bcps[:])
                            a_t = atp.tile([128, 512], BF16, name=f"at{qc}_{h}", tag=f"at{h}")
                            nc.vector.tensor_mul(a_t[:], pvs[h][:], bc_sb[:])
                            at_t[(qc, h)] = a_t
                    weave_step(128)
                    scope.__exit__(None, None, None)

                attn_chunk(0)
                attn_chunk(1, weave=0)
                attn_chunk(2, weave=1)
                attn_chunk(3, weave=2)
                scope = nc.named_scope("wo3"); scope.__enter__()
                for m in range(128):
                    wo_mm(3, m)
                scope.__exit__(None, None, None)

    _split_multi_waits(nc)
    return nc


def _host_prep(x, cos, sin, wq, wk, wv, wo):
    scale = np.float32(HD ** -0.5)
    perm = np.concatenate([np.arange(0, HD, 2), np.arange(1, HD, 2)])

    xT = np.ascontiguousarray(x.T.astype(_BF16))
    cosT = np.ascontiguousarray(cos.T)
    sinT = np.ascontiguousarray(sin.T)
    cs1 = np.concatenate([cosT, -sinT], axis=0)
    cs2 = np.concatenate([sinT, cosT], axis=0)

    shared = {
        "xt": xT,
        "cs1": cs1,
        "cs2": cs2,
        "onesc": np.ones((HD, 1), np.float32).astype(_BF16),
        "onesr": np.ones((1, HD), np.float32),
        "ident": np.eye(HD, dtype=np.float32).astype(_BF16),
    }
    in_maps = []
    for c in range(N_CORES):
        wq_c = wq[c * 512:(c + 1) * 512].reshape(N_QH, HD, D)[:, perm, :]
        wq_c = (wq_c.reshape(512, D) * scale)
        wk_c = wk[c * HD:(c + 1) * HD][perm, :]
        wv_c = wv[c * HD:(c + 1) * HD]
        # woT for the local-wo GEMM: [p, ct, n] with chan = c*512 + ct*128 + p
        wo_c = np.ascontiguousarray(
            wo[:, c * 512:(c + 1) * 512].T.reshape(N_QH, 128, D).transpose(1, 0, 2)
        )
        m = dict(shared)
        m["wqt"] = np.ascontiguousarray(wq_c.T).astype(_BF16)
        m["wkt"] = np.ascontiguousarray(wk_c.T).astype(_BF16)
        m["wvt"] = np.ascontiguousarray(wv_c.T).astype(_BF16)
        m["wot"] = wo_c.astype(_BF16)
        in_maps.append(m)
    return in_maps


def kernel(x, cos, sin, wq, wk, wv, wo, _trace=False):
    x = np.asarray(x, np.float32)
    cos = np.asarray(cos, np.float32)
    sin = np.asarray(sin, np.float32)
    wq = np.asarray(wq, np.float32)
    wk = np.asarray(wk, np.float32)
    wv = np.asarray(wv, np.float32)
    wo = np.asarray(wo, np.float32)

    in_maps = _host_prep(x, cos, sin, wq, wk, wv, wo)
    if "nc" not in _NC_CACHE:
        _NC_CACHE["nc"] = _build()
    nc = _NC_CACHE["nc"]
    res = run_bass_kernel_spmd(
        nc, in_maps, core_ids=list(range(N_CORES)), trace=_trace
    )
    out = np.concatenate(
        [np.asarray(res.results[c]["out"]) for c in range(N_CORES)], axis=1
    )
    out = np.ascontiguousarray(out.astype(np.float32))
    if _trace:
        kernel._last_exec_time_ns = res.exec_time_ns
        kernel._last_result = res
    return out


# revision 18
# speedup vs baseline: 1.2112x; 1.0677x over previous
"""GQA attention (S=2048, D=4096, 32 Q heads / 8 KV heads, RoPE, full attn)
distributed over 8 Trainium2 NeuronCores.

Strategy (tensor-parallel by heads, local-wo + ReduceScatter):
  - core c owns Q heads 4c..4c+3 and KV head c (GQA groups align with cores).
  - projections as transposed GEMMs: QT/KT [chan, tok] directly usable by
    the scores matmul; V via VT + PE transposes; all big matmuls bf16
    (Fast Weight Load), f32 PSUM accumulate.
  - RoPE on the DVE only: with host-deinterleaved channels, r1 = u_lo - u_hi
    and r2 = v_lo + v_hi where u = src*cs1, v = src*cs2 (partition-split
    operands; PSUM src + SBUF table makes the mixed-base read legal).
  - scores transposed, ST = KT.T @ QT -> [k, q]; exp on ScalarE (bf16 out);
    softmax normalizer: bf16 DVE partial sums -> ones-matmul -> ln/exp on
    ScalarE -> f32r broadcast-matmul -> DVE scale.
  - output projection is LOCAL: out_partial[tok, 4096] = at_local.T @ woT
    (stationary = attention tile, moving = woT blocks), quantized bf16 and
    ReduceScattered per q-chunk; rank c's shard is exactly its out columns.
    No AllGather, no gather staging, and wo never waits on a collective.
  - wo(qc) matmuls are WOVEN into attn(qc+1)'s kt loop (4 per slot): the PE
    queue never idles while the scalar engine streams exps.
  - a full-sized dummy ReduceScatter during the projections absorbs the
    ncfw cold-start + RDH algorithm first-use cost (~60us -> ~25us).
  - consecutive matmuls that reuse the same bf16 stationary operand get
    ldweights=False patched post-schedule (skips the ~50ns serialized
    weight load; scores/PV pairs, wo jo-pairs, zps pairs).

Host side only reshapes/transposes/pads/casts and concatenates outputs
(final bf16 -> f32 upcast included).
"""
import sys

import numpy as np
import ml_dtypes

_BF16 = ml_dtypes.bfloat16

for _p in ("/root/.axon_site/_ro/trn_rl_repo", "/opt/trn_rl_repo"):
    if _p not in sys.path:
        sys.path.append(_p)

import concourse.bass as bass
import concourse.tile as tile
from concourse import mybir
from concourse.bass_utils import run_bass_kernel_spmd

N_CORES = 8
S = 2048
D = 4096
HD = 128
N_QH = 4          # Q heads per core
N_KT = S // 128   # 16 k-tiles
N_TC = S // 512   # 4 token chunks
N_KC = D // 128   # 32 contraction tiles
F32 = mybir.dt.float32
F32R = mybir.dt.float32r
BF16 = mybir.dt.bfloat16

_NC_CACHE = {}


def _split_multi_waits(nc):
    """This container's walrus accepts only ONE sync-wait per instruction
    encoding; hoist extra waits onto fresh single-wait NoOps placed before
    the instruction on the same engine."""
    n = 0
    for fn in nc.m.functions:
        for bb in fn.blocks:
            new_insts = []
            changed = False
            for ins in bb.instructions:
                si = ins.sync_info
                waits = list(si.on_wait) if si is not None else []
                if len(waits) > 1:
                    for w in waits[:-1]:
                        n += 1
                        nop = mybir.InstNoOp(name=f"WSPL-{n}", ins=[], outs=[])
                        nop.engine = ins.engine
                        nop.sync_info = mybir.SyncInfo(on_wait=[w], on_update=[])
                        new_insts.append(nop)
                    si.on_wait = waits[-1:]
                    changed = True
                new_insts.append(ins)
            if changed:
                bb.instructions = new_insts
    return n


def _dedup_ldweights(nc):
    """Consecutive matmuls (no other tensor-engine instruction between)
    with an identical bf16 stationary operand: the later ones reuse the
    already-loaded PE array weights (ldweights=False). f32/f32r stationary
    is excluded (non-self-loading f32r matmuls return zeros on HW)."""
    n = 0
    for fn in nc.m.functions:
        for bb in fn.blocks:
            prev = None  # (engine, weights_repr) of last tensor-engine inst
            for ins in bb.instructions:
                if isinstance(ins, mybir.InstMatmult):
                    if ins.is_transpose:
                        prev = None
                        continue
                    w = ins.ins[1]
                    wrep = str(w)
                    ok_dtype = "bfloat16" in wrep
                    if (
                        prev is not None
                        and wrep == prev
                        and ok_dtype
                        and ins.perf_mode is None
                    ):
                        ins.ldweights = False
                        n += 1
                    prev = wrep
                elif isinstance(ins, mybir.InstLdweights):
                    prev = None
                # other engines' instructions don't touch PE weights
    return n


def _build():
    nc = bass.Bass()

    xt = nc.dram_tensor("xt", [D, S], BF16, kind="ExternalInput")
    wqt = nc.dram_tensor("wqt", [D, 512], BF16, kind="ExternalInput")
    wkt = nc.dram_tensor("wkt", [D, HD], BF16, kind="ExternalInput")
    wvt = nc.dram_tensor("wvt", [D, HD], BF16, kind="ExternalInput")
    wot = nc.dram_tensor("wot", [128, N_QH, D], BF16, kind="ExternalInput")
    cs1 = nc.dram_tensor("cs1", [HD, S], F32, kind="ExternalInput")
    cs2 = nc.dram_tensor("cs2", [HD, S], F32, kind="ExternalInput")
    onesc = nc.dram_tensor("onesc", [HD, 1], BF16, kind="ExternalInput")
    onesr = nc.dram_tensor("onesr", [1, HD], F32R, kind="ExternalInput")
    ident = nc.dram_tensor("ident", [HD, HD], BF16, kind="ExternalInput")
    out_ext = nc.dram_tensor("out", [S, 512], BF16, kind="ExternalOutput")

    # per-chunk ReduceScatter buffers: rs_in[b, t, n] holds the partial
    # contribution to out columns b*512..(b+1)*512 -> rank b receives the
    # b-th contiguous shard of the sum = its own out columns. The last
    # chunk is split into two token halves so the first RS overlaps the
    # second half of wo3 (shorter serial tail).
    rs_in = [
        nc.dram_tensor(f"rsi{qc}", [N_CORES, 512, 512], BF16) for qc in range(3)
    ] + [
        nc.dram_tensor(f"rsi3{h}", [N_CORES, 256, 512], BF16) for h in range(2)
    ]
    rs_out = [
        nc.dram_tensor(f"rso{qc}", [512, 512], BF16) for qc in range(3)
    ] + [
        nc.dram_tensor(f"rso3{h}", [256, 512], BF16) for h in range(2)
    ]
    # full-sized dummy RS fired during the projections: absorbs the ncfw
    # cold-start AND the RDH-algorithm first-use cost so RS0 runs hot.
    warm_in = nc.dram_tensor("warmi", [N_CORES, 512, 512], BF16)
    warm_out = nc.dram_tensor("warmo", [512, 512], BF16)

    xt_r = xt.rearrange("(kc p) s -> kc p s", p=128)
    wqt_r = wqt.rearrange("(kc p) n -> kc p n", p=128)
    wkt_r = wkt.rearrange("(kc p) n -> kc p n", p=128)
    wvt_r = wvt.rearrange("(kc p) n -> kc p n", p=128)


    with tile.TileContext(nc) as tc:
        with (
            tc.tile_pool(name="const", bufs=1) as constp,
            tc.tile_pool(name="persist", bufs=1) as persist,
        ):
            onesc_sb = constp.tile([HD, 1], BF16)
            onesr_sb = constp.tile([1, HD], F32R)

            qt_sb = persist.tile([128, N_QH, S], BF16)   # QT_rope
            kt_sb = persist.tile([128, S], BF16)         # KT_rope
            v_sb = persist.tile([128, N_KT, HD], BF16)   # V [tok-in-tile, kt, chan]

            # ---------------- phase 1: projections + rope ----------------
            with (
                tc.tile_pool(name="wq", bufs=1) as wqp,
                tc.tile_pool(name="csp", bufs=1) as csp,
                tc.tile_pool(name="xtp", bufs=3) as xtp,
                tc.tile_pool(name="uv", bufs=2) as uvp,
                tc.tile_pool(name="vt", bufs=2) as vtp,
                tc.tile_pool(name="p1q", bufs=1, space="PSUM") as p1q,
                tc.tile_pool(name="p1k", bufs=1, space="PSUM") as p1k,
                tc.tile_pool(name="p1r", bufs=1, space="PSUM") as p1r,
            ):
                wq_sb = wqp.tile([128, N_KC, 512], BF16)
                wk_sb = wqp.tile([128, N_KC, HD], BF16)
                wv_sb = wqp.tile([128, N_KC, HD], BF16)
                cs1_sb = csp.tile([HD, S], F32)
                cs2_sb = csp.tile([HD, S], F32)
                ident_sb = csp.tile([HD, HD], BF16)

                nc.gpsimd.collective_compute(
                    "ReduceScatter",
                    mybir.AluOpType.add,
                    replica_groups=[list(range(N_CORES))],
                    ins=[warm_in[:].opt()],
                    outs=[warm_out[:].opt()],
                )
                # weights in 8-kc groups, interleaved wq/wk/wv so the kc
                # loop's consumption never outruns the DMA supply.
                for ch in range(4):
                    nc.gpsimd.dma_start(
                        out=wq_sb[:, ch * 8:(ch + 1) * 8, :],
                        in_=wqt_r[ch * 8:(ch + 1) * 8].rearrange("kc p n -> p kc n"),
                    )
                    nc.gpsimd.dma_start(
                        out=wk_sb[:, ch * 8:(ch + 1) * 8, :],
                        in_=wkt_r[ch * 8:(ch + 1) * 8].rearrange("kc p n -> p kc n"),
                    )
                    nc.gpsimd.dma_start(
                        out=wv_sb[:, ch * 8:(ch + 1) * 8, :],
                        in_=wvt_r[ch * 8:(ch + 1) * 8].rearrange("kc p n -> p kc n"),
                    )
                nc.gpsimd.dma_start(out=cs1_sb[:], in_=cs1[:])
                nc.gpsimd.dma_start(out=cs2_sb[:], in_=cs2[:])
                nc.gpsimd.dma_start(out=onesc_sb[:], in_=onesc[:])
                nc.gpsimd.dma_start(out=onesr_sb[:], in_=onesr[:])
                nc.gpsimd.dma_start(out=ident_sb[:], in_=ident[:])

                for tcb in range(N_TC):
                    t0 = tcb * 512
                    scope = nc.named_scope(f"proj{tcb}"); scope.__enter__()
                    qps = [
                        p1q.tile([128, 512], F32, name=f"qps{tcb}_{h}", tag=f"qps{h}")
                        for h in range(N_QH)
                    ]
                    kps = p1k.tile([128, 512], F32, name=f"kps{tcb}", tag="kps")
                    vtps = p1k.tile([128, 512], F32, name=f"vtps{tcb}", tag="vtps")
                    xt_g = None
                    for kc in range(N_KC):
                        if kc % 8 == 0:
                            xt_g = xtp.tile([128, 8, 512], BF16, name=f"xt{tcb}_{kc}", tag="xt")
                            nc.sync.dma_start(
                                out=xt_g[:],
                                in_=xt_r[kc:kc + 8, :, t0:t0 + 512].rearrange("g p n -> p g n"),
                            )
                        xt_t = xt_g[:, kc % 8, :]
                        st, sp = kc == 0, kc == N_KC - 1
                        for h in range(N_QH):
                            nc.tensor.matmul(
                                qps[h][:], wq_sb[:, kc, h * 128:(h + 1) * 128],
                                xt_t, start=st, stop=sp,
                            )
                        nc.tensor.matmul(kps[:], wk_sb[:, kc, :], xt_t, start=st, stop=sp)
                        nc.tensor.matmul(vtps[:], wv_sb[:, kc, :], xt_t, start=st, stop=sp)

                    # rope on DVE: K first (attention depends on full KT).
                    # With deinterleaved chans, cs1=[cos;-sin], cs2=[sin;cos]:
                    #   a = [v1*c ; v1*s] (both from src_lo),
                    #   b = [-v2*s ; v2*c] (both from src_hi),  r = a + b.
                    # Every mul keeps PSUM-in base == SBUF-in base (legal);
                    # only the out base differs. The add is one full-lane op.
                    for h in [N_QH] + list(range(N_QH)):
                        src = kps if h == N_QH else qps[h]
                        a_t = uvp.tile([128, 512], BF16, name=f"u{tcb}_{h}", tag="u")
                        b_t = uvp.tile([128, 512], BF16, name=f"v{tcb}_{h}", tag="v")
                        nc.vector.tensor_mul(a_t[0:64, :], src[0:64, :], cs1_sb[0:64, t0:t0 + 512])
                        nc.vector.tensor_mul(a_t[64:128, :], src[0:64, :], cs2_sb[0:64, t0:t0 + 512])
                        nc.vector.tensor_mul(b_t[0:64, :], src[64:128, :], cs1_sb[64:128, t0:t0 + 512])
                        nc.vector.tensor_mul(b_t[64:128, :], src[64:128, :], cs2_sb[64:128, t0:t0 + 512])
                        if h == N_QH:
                            dst = kt_sb[:, t0:t0 + 512]
                        else:
                            dst = qt_sb[:, h, t0:t0 + 512]
                        nc.vector.tensor_add(dst, a_t[:], b_t[:])

                    # V for this token chunk: VT -> PE transpose -> V
                    vt_sb = vtp.tile([128, 512], BF16, name=f"vts{tcb}", tag="vts")
                    nc.vector.tensor_copy(vt_sb[:], vtps[:])
                    vtr = p1r.tile([128, 4, 128], BF16, name=f"vtr{tcb}", tag="vtr")
                    for j in range(4):
                        nc.tensor.transpose(
                            vtr[:, j, :], vt_sb[:, j * 128:(j + 1) * 128],
                            ident_sb[:],
                        )
                    nc.vector.tensor_copy(v_sb[:, tcb * 4:(tcb + 1) * 4, :], vtr[:])
                    scope.__exit__(None, None, None)

            # -- phase 2: attention, with prev chunk's local wo GEMM woven in --
            with (
                tc.tile_pool(name="wo", bufs=1) as wop,
                tc.tile_pool(name="ep", bufs=6) as ep,
                tc.tile_pool(name="zp", bufs=1) as zp,
                tc.tile_pool(name="np_", bufs=2) as np_,
                tc.tile_pool(name="atp", bufs=2) as atp,
                tc.tile_pool(name="rsev", bufs=8) as rsev,
                tc.tile_pool(name="p2s", bufs=2, space="PSUM") as p2s,
                tc.tile_pool(name="p2pv", bufs=1, space="PSUM") as p2pv,
                tc.tile_pool(name="p2wo", bufs=1, space="PSUM") as p2wo,
            ):
                wo_sb = wop.tile([128, N_QH, D], BF16)
                nc.gpsimd.dma_start(out=wo_sb[:], in_=wot[:])

                at_t = {}      # (qc, h) -> normalized attention tile [128, 512]
                wo_wps = {}

                def fire_rs(ri, tok0, ntok):
                    # rs_in[ri] is complete: reduce-scatter it and copy this
                    # rank's shard to the output rows. Everything here lives
                    # on the gpsimd queue: it waits for the collective, but
                    # nothing latency-critical queues behind it there (the
                    # sync queue must stay free for the next chunk's rs_in
                    # writes or the whole weave wedges on buffer recycling).
                    sc2 = nc.named_scope(f"rs{ri}"); sc2.__enter__()
                    nc.gpsimd.collective_compute(
                        "ReduceScatter",
                        mybir.AluOpType.add,
                        replica_groups=[list(range(N_CORES))],
                        ins=[rs_in[ri][:].opt()],
                        outs=[rs_out[ri][:].opt()],
                    )
                    if tok0 is not None:
                        nc.gpsimd.dma_start(
                            out=out_ext[tok0:tok0 + ntok, :], in_=rs_out[ri][:]
                        )
                    sc2.__exit__(None, None, None)

                def wo_mm(qc, m):
                    # one wo matmul: m = ts*32 + q4*8 + ct*2 + jo
                    # out_partial[ts-tile, ob*512:+512] += at[ct][:,ts].T @ woT[ct, ob]
                    ts, q4 = m // 32, (m % 32) // 8
                    r = m % 8
                    ct, jo = r // 2, r % 2
                    if r == 0:
                        wo_wps[qc] = p2wo.tile(
                            [128, 2, 512], F32, name=f"wps{qc}_{m}", tag="wo"
                        )
                    wps = wo_wps[qc]
                    ob = q4 * 2 + jo
                    nc.tensor.matmul(
                        wps[:, jo, :],
                        at_t[(qc, ct)][:, ts * 128:(ts + 1) * 128],
                        wo_sb[:, ct, ob * 512:(ob + 1) * 512],
                        start=(ct == 0), stop=(ct == 3),
                    )
                    if r == 7:
                        ev = rsev.tile(
                            [128, 2, 512], BF16, name=f"wev{qc}_{m}", tag="wev"
                        )
                        nc.vector.tensor_copy(ev[:], wps[:])
                        if qc < 3:
                            ri, row = qc, ts * 128
                        else:
                            ri, row = 3 + ts // 2, (ts % 2) * 128
                        nc.sync.dma_start(
                            out=rs_in[ri][q4 * 2:q4 * 2 + 2, row:row + 128, :]
                            .rearrange("j p n -> p j n"),
                            in_=ev[:],
                        )
                    if qc < 3:
                        if m == 127:
                            fire_rs(qc, qc * 512, 512)
                    else:
                        # triggers first, rank-shard copies after: an out
                        # copy waiting on RS3a must not delay RS3b's launch.
                        if m == 63:
                            fire_rs(3, None, None)
                        elif m == 127:
                            fire_rs(4, None, None)
                            for ri2, (t0o, nt) in ((3, (1536, 256)), (4, (1792, 256))):
                                nc.gpsimd.dma_start(
                                    out=out_ext[t0o:t0o + nt, :], in_=rs_out[ri2][:]
                                )

                def attn_chunk(qc, weave=None):
                    # weave: chunk index whose wo matmuls (128) are spread
                    # over this chunk's 32 kt slots, 4 per slot.
                    q0 = qc * 512
                    scope = nc.named_scope(f"attn{qc}"); scope.__enter__()
                    wv_n = [0]

                    def weave_step(upto):
                        if weave is None:
                            return
                        while wv_n[0] < min(upto, 128):
                            wo_mm(weave, wv_n[0])
                            wv_n[0] += 1

                    for hp in range(2):
                        hs = [2 * hp, 2 * hp + 1]
                        pvs = {
                            h: p2pv.tile([128, 512], F32, name=f"pv{qc}_{h}", tag=f"pv{h % 2}")
                            for h in hs
                        }
                        zparts = {
                            h: zp.tile([128, 512], BF16, name=f"zpt{qc}_{h}", tag=f"zpart{h % 2}")
                            for h in hs
                        }
                        for kt in range(N_KT):
                            k0 = kt * 128
                            stp = p2s.tile([128, 2, 512], F32, name=f"st{qc}_{hp}_{kt}", tag="st")
                            for j, h in enumerate(hs):
                                nc.tensor.matmul(
                                    stp[:, j, :], kt_sb[:, k0:k0 + 128],
                                    qt_sb[:, h, q0:q0 + 512], start=True, stop=True,
                                )
                            e_t = ep.tile([128, 2, 512], BF16, name=f"e{qc}_{kt}_{hp}", tag="e")
                            nc.scalar.activation(
                                out=e_t[:], in_=stp[:],
                                func=mybir.ActivationFunctionType.Exp,
                            )
                            for j, h in enumerate(hs):
                                if kt == 0:
                                    nc.vector.tensor_copy(zparts[h][:], e_t[:, j, :])
                                else:
                                    nc.vector.tensor_add(zparts[h][:], zparts[h][:], e_t[:, j, :])
                            for j, h in enumerate(hs):
                                nc.tensor.matmul(
                                    pvs[h][:], v_sb[:, kt, :], e_t[:, j, :],
                                    start=(kt == 0), stop=(kt == N_KT - 1),
                                )
                            weave_step((hp * N_KT + kt + 1) * 4)
                        zpss = {}
                        for h in hs:
                            zpss[h] = p2s.tile([1, 512], F32, name=f"zps{qc}_{h}", tag="st")
                            nc.tensor.matmul(zpss[h][:], onesc_sb[:], zparts[h][:], start=True, stop=True)
                        invzs = {}
                        for h in hs:
                            lnz = zp.tile([1, 512], F32, name=f"lnz{qc}_{h}", tag=f"lnz{h % 2}")
                            nc.scalar.activation(
                                out=lnz[:], in_=zpss[h][:],
                                func=mybir.ActivationFunctionType.Ln,
                            )
                            invzs[h] = zp.tile([1, 512], F32R, name=f"izr{qc}_{h}", tag=f"invz{h % 2}")
                            nc.scalar.activation(
                                out=invzs[h][:], in_=lnz[:],
                                func=mybir.ActivationFunctionType.Exp, scale=-1.0,
                            )
                        for h in hs:
                            bcps = p2s.tile([128, 512], F32, name=f"bc{qc}_{h}", tag="st")
                            nc.tensor.matmul(bcps[:], onesr_sb[:], invzs[h][:], start=True, stop=True)
                            bc_sb = np_.tile([128, 512], F32, name=f"bcs{qc}_{h}", tag="bcs")
                            nc.vector.tensor_copy(bc_sb[:], bcps[:])
                            a_t = atp.tile([128, 512], BF16, name=f"at{qc}_{h}", tag=f"at{h}")
                            nc.vector.tensor_mul(a_t[:], pvs[h][:], bc_sb[:])
                            at_t[(qc, h)] = a_t
                    weave_step(128)
                    scope.__exit__(None, None, None)

                attn_chunk(0)
                attn_chunk(1, weave=0)
                attn_chunk(2, weave=1)
                attn_chunk(3, weave=2)
                scope = nc.named_scope("wo3"); scope.__enter__()
                for m in range(128):
                    wo_mm(3, m)
                scope.__exit__(None, None, None)

    _split_multi_waits(nc)
    return nc


def _host_prep(x, cos, sin, wq, wk, wv, wo):
    scale = np.float32(HD ** -0.5)
    perm = np.concatenate([np.arange(0, HD, 2), np.arange(1, HD, 2)])

    xT = np.ascontiguousarray(x.T.astype(_BF16))
    cosT = np.ascontiguousarray(cos.T)
    sinT = np.ascontiguousarray(sin.T)
    cs1 = np.concatenate([cosT, -sinT], axis=0)
    cs2 = np.concatenate([sinT, cosT], axis=0)

    shared = {
        "xt": xT,
        "cs1": cs1,
        "cs2": cs2,
        "onesc": np.ones((HD, 1), np.float32).astype(_BF16),
        "onesr": np.ones((1, HD), np.float32),
        "ident": np.eye(HD, dtype=np.float32).astype(_BF16),
    }
    in_maps = []
    for c in range(N_CORES):
        wq_c = wq[c * 512:(c + 1) * 512].reshape(N_QH, HD, D)[:, perm, :]
        wq_c = (wq_c.reshape(512, D) * scale)
        wk_c = wk[c * HD:(c + 1) * HD][perm, :]
        wv_c = wv[c * HD:(c + 1) * HD]
        # woT for the local-wo GEMM: [p, ct, n] with chan = c*512 + ct*128 + p
        wo_c = np.ascontiguousarray(
            wo[:, c * 512:(c + 1) * 512].T.reshape(N_QH, 128, D).transpose(1, 0, 2)
        )
        m = dict(shared)
        m["wqt"] = np.ascontiguousarray(wq_c.T).astype(_BF16)
        m["wkt"] = np.ascontiguousarray(wk_c.T).astype(_BF16)
        m["wvt"] = np.ascontiguousarray(wv_c.T).astype(_BF16)
        m["wot"] = wo_c.astype(_BF16)
        in_maps.append(m)
    return in_maps


def kernel(x, cos, sin, wq, wk, wv, wo, _trace=False):
    x = np.asarray(x, np.float32)
    cos = np.asarray(cos, np.float32)
    sin = np.asarray(sin, np.float32)
    wq = np.asarray(wq, np.float32)
    wk = np.asarray(wk, np.float32)
    wv = np.asarray(wv, np.float32)
    wo = np.asarray(wo, np.float32)

    in_maps = _host_prep(x, cos, sin, wq, wk, wv, wo)
    if "nc" not in _NC_CACHE:
        _NC_CACHE["nc"] = _build()
    nc = _NC_CACHE["nc"]
    res = run_bass_kernel_spmd(
        nc, in_maps, core_ids=list(range(N_CORES)), trace=_trace
    )
    out = np.concatenate(
        [np.asarray(res.results[c]["out"]) for c in range(N_CORES)], axis=1
    )
    out = np.ascontiguousarray(out.astype(np.float32))
    if _trace:
        kernel._last_exec_time_ns = res.exec_time_ns
        kernel._last_result = res
    return out


# revision 26
# speedup vs baseline: 1.2373x; 1.0216x over previous
"""GQA attention (S=2048, D=4096, 32 Q heads / 8 KV heads, RoPE, full attn)
distributed over 8 Trainium2 NeuronCores.

Strategy (tensor-parallel by heads, local-wo + ReduceScatter):
  - core c owns Q heads 4c..4c+3 and KV head c (GQA groups align with cores).
  - projections as transposed GEMMs: QT/KT [chan, tok] directly usable by
    the scores matmul; V via VT + PE transposes; all big matmuls bf16
    (Fast Weight Load), f32 PSUM accumulate.
  - RoPE on the DVE only: with host-deinterleaved channels, r1 = u_lo - u_hi
    and r2 = v_lo + v_hi where u = src*cs1, v = src*cs2 (partition-split
    operands; PSUM src + SBUF table makes the mixed-base read legal).
  - scores transposed, ST = KT.T @ QT -> [k, q]; exp on ScalarE (bf16 out);
    softmax normalizer: bf16 DVE partial sums -> ones-matmul -> ln/exp on
    ScalarE -> f32r broadcast-matmul -> DVE scale.
  - output projection is LOCAL: out_partial[tok, 4096] = at_local.T @ woT
    (stationary = attention tile, moving = woT blocks), quantized bf16 and
    ReduceScattered per q-chunk; rank c's shard is exactly its out columns.
    No AllGather, no gather staging, and wo never waits on a collective.
  - wo(qc) matmuls are WOVEN into attn(qc+1)'s kt loop (4 per slot): the PE
    queue never idles while the scalar engine streams exps.
  - a full-sized dummy ReduceScatter during the projections absorbs the
    ncfw cold-start + RDH algorithm first-use cost (~60us -> ~25us).
  - consecutive matmuls that reuse the same bf16 stationary operand get
    ldweights=False patched post-schedule (skips the ~50ns serialized
    weight load; scores/PV pairs, wo jo-pairs, zps pairs).

Host side only reshapes/transposes/pads/casts and concatenates outputs
(final bf16 -> f32 upcast included).
"""
import sys

import numpy as np
import ml_dtypes

_BF16 = ml_dtypes.bfloat16

for _p in ("/root/.axon_site/_ro/trn_rl_repo", "/opt/trn_rl_repo"):
    if _p not in sys.path:
        sys.path.append(_p)

import concourse.bass as bass
import concourse.tile as tile
from concourse import mybir
from concourse.bass_utils import run_bass_kernel_spmd

N_CORES = 8
S = 2048
D = 4096
HD = 128
N_QH = 4          # Q heads per core
N_KT = S // 128   # 16 k-tiles
N_TC = S // 512   # 4 token chunks
N_KC = D // 128   # 32 contraction tiles
F32 = mybir.dt.float32
F32R = mybir.dt.float32r
BF16 = mybir.dt.bfloat16

_NC_CACHE = {}


def _split_multi_waits(nc):
    """This container's walrus accepts only ONE sync-wait per instruction
    encoding; hoist extra waits onto fresh single-wait NoOps placed before
    the instruction on the same engine."""
    n = 0
    for fn in nc.m.functions:
        for bb in fn.blocks:
            new_insts = []
            changed = False
            for ins in bb.instructions:
                si = ins.sync_info
                waits = list(si.on_wait) if si is not None else []
                if len(waits) > 1:
                    for w in waits[:-1]:
                        n += 1
                        nop = mybir.InstNoOp(name=f"WSPL-{n}", ins=[], outs=[])
                        nop.engine = ins.engine
                        nop.sync_info = mybir.SyncInfo(on_wait=[w], on_update=[])
                        new_insts.append(nop)
                    si.on_wait = waits[-1:]
                    changed = True
                new_insts.append(ins)
            if changed:
                bb.instructions = new_insts
    return n


def _dedup_ldweights(nc):
    """Consecutive matmuls (no other tensor-engine instruction between)
    with an identical bf16 stationary operand: the later ones reuse the
    already-loaded PE array weights (ldweights=False). f32/f32r stationary
    is excluded (non-self-loading f32r matmuls return zeros on HW)."""
    n = 0
    for fn in nc.m.functions:
        for bb in fn.blocks:
            prev = None  # (engine, weights_repr) of last tensor-engine inst
            for ins in bb.instructions:
                if isinstance(ins, mybir.InstMatmult):
                    if ins.is_transpose:
                        prev = None
                        continue
                    w = ins.ins[1]
                    wrep = str(w)
                    ok_dtype = "bfloat16" in wrep
                    if (
                        prev is not None
                        and wrep == prev
                        and ok_dtype
                        and ins.perf_mode is None
                    ):
                        ins.ldweights = False
                        n += 1
                    prev = wrep
                elif isinstance(ins, mybir.InstLdweights):
                    prev = None
                # other engines' instructions don't touch PE weights
    return n


def _build():
    nc = bass.Bass()

    xt = nc.dram_tensor("xt", [D, S], BF16, kind="ExternalInput")
    wqt = nc.dram_tensor("wqt", [D, 512], BF16, kind="ExternalInput")
    wkt = nc.dram_tensor("wkt", [D, HD], BF16, kind="ExternalInput")
    wvt = nc.dram_tensor("wvt", [D, HD], BF16, kind="ExternalInput")
    wot = nc.dram_tensor("wot", [128, N_QH, D], BF16, kind="ExternalInput")
    cs1 = nc.dram_tensor("cs1", [HD, S], F32, kind="ExternalInput")
    cs2 = nc.dram_tensor("cs2", [HD, S], F32, kind="ExternalInput")
    onesc = nc.dram_tensor("onesc", [HD, 1], BF16, kind="ExternalInput")
    onesr = nc.dram_tensor("onesr", [1, HD], F32R, kind="ExternalInput")
    ident = nc.dram_tensor("ident", [HD, HD], BF16, kind="ExternalInput")
    out_ext = nc.dram_tensor("out", [S, 512], BF16, kind="ExternalOutput")

    # ReduceScatter buffers, one per 256-token half-chunk: rs_in[b, t, n]
    # holds the partial contribution to out columns b*512..(b+1)*512 ->
    # rank b receives the b-th contiguous shard of the sum = its own out
    # columns. Half-chunks keep each CC op ~26us so the tail pair drains
    # fast and the stream never backs up.
    rs_in = [
        nc.dram_tensor(f"rsi{i}", [N_CORES, 256, 512], BF16) for i in range(8)
    ]
    rs_out = [
        nc.dram_tensor(f"rso{i}", [256, 512], BF16) for i in range(8)
    ]
    # same-shape dummy RS fired during the projections: absorbs the ncfw
    # cold-start AND the algorithm first-use cost so RS0 runs hot.
    warm_in = nc.dram_tensor("warmi", [N_CORES, 256, 512], BF16)
    warm_out = nc.dram_tensor("warmo", [256, 512], BF16)

    xt_r = xt.rearrange("(kc p) s -> kc p s", p=128)
    wqt_r = wqt.rearrange("(kc p) n -> kc p n", p=128)
    wkt_r = wkt.rearrange("(kc p) n -> kc p n", p=128)
    wvt_r = wvt.rearrange("(kc p) n -> kc p n", p=128)


    with tile.TileContext(nc) as tc:
        with (
            tc.tile_pool(name="const", bufs=1) as constp,
            tc.tile_pool(name="persist", bufs=1) as persist,
        ):
            onesc_sb = constp.tile([HD, 1], BF16)
            onesr_sb = constp.tile([1, HD], F32R)

            # per-token-chunk tiles (NOT one big tile): Tile's dependency
            # tracking is per-tile, so attn chunk 0's first scores must not
            # falsely wait on proj3's rope writes.
            qtt = [
                persist.tile([128, N_QH, 512], BF16, name=f"qtt{i}")
                for i in range(N_TC)
            ]
            ktt = [
                persist.tile([128, 512], BF16, name=f"ktt{i}")
                for i in range(N_TC)
            ]
            v_sb = persist.tile([128, N_KT, HD], BF16)   # V [tok-in-tile, kt, chan]

            # ---------------- phase 1: projections + rope ----------------
            with (
                tc.tile_pool(name="wq", bufs=1) as wqp,
                tc.tile_pool(name="csp", bufs=1) as csp,
                tc.tile_pool(name="xtp", bufs=3) as xtp,
                tc.tile_pool(name="uv", bufs=2) as uvp,
                tc.tile_pool(name="vt", bufs=2) as vtp,
                tc.tile_pool(name="p1q", bufs=1, space="PSUM") as p1q,
                tc.tile_pool(name="p1k", bufs=1, space="PSUM") as p1k,
                tc.tile_pool(name="p1r", bufs=1, space="PSUM") as p1r,
            ):
                wq_sb = wqp.tile([128, N_KC, 512], BF16)
                wk_sb = wqp.tile([128, N_KC, HD], BF16)
                wv_sb = wqp.tile([128, N_KC, HD], BF16)
                cs1_sb = csp.tile([HD, S], F32)
                cs2_sb = csp.tile([HD, S], F32)
                ident_sb = csp.tile([HD, HD], BF16)

                nc.gpsimd.collective_compute(
                    "ReduceScatter",
                    mybir.AluOpType.add,
                    replica_groups=[list(range(N_CORES))],
                    ins=[warm_in[:].opt()],
                    outs=[warm_out[:].opt()],
                )
                # first 8-kc weight group on the fast sync queue (ahead of
                # the xt loads the proj loop enqueues) so the first matmul
                # starts ~5us in; the rest stream on gpsimd.
                nc.sync.dma_start(
                    out=wq_sb[:, 0:8, :],
                    in_=wqt_r[0:8].rearrange("kc p n -> p kc n"),
                )
                nc.sync.dma_start(
                    out=wk_sb[:, 0:8, :],
                    in_=wkt_r[0:8].rearrange("kc p n -> p kc n"),
                )
                nc.sync.dma_start(
                    out=wv_sb[:, 0:8, :],
                    in_=wvt_r[0:8].rearrange("kc p n -> p kc n"),
                )
                for ch in (1, 2, 3):
                    nc.gpsimd.dma_start(
                        out=wq_sb[:, ch * 8:(ch + 1) * 8, :],
                        in_=wqt_r[ch * 8:(ch + 1) * 8].rearrange("kc p n -> p kc n"),
                    )
                    nc.gpsimd.dma_start(
                        out=wk_sb[:, ch * 8:(ch + 1) * 8, :],
                        in_=wkt_r[ch * 8:(ch + 1) * 8].rearrange("kc p n -> p kc n"),
                    )
                    nc.gpsimd.dma_start(
                        out=wv_sb[:, ch * 8:(ch + 1) * 8, :],
                        in_=wvt_r[ch * 8:(ch + 1) * 8].rearrange("kc p n -> p kc n"),
                    )
                nc.gpsimd.dma_start(out=cs1_sb[:], in_=cs1[:])
                nc.gpsimd.dma_start(out=cs2_sb[:], in_=cs2[:])
                nc.gpsimd.dma_start(out=onesc_sb[:], in_=onesc[:])
                nc.gpsimd.dma_start(out=onesr_sb[:], in_=onesr[:])
                nc.gpsimd.dma_start(out=ident_sb[:], in_=ident[:])

                for tcb in range(N_TC):
                    t0 = tcb * 512
                    scope = nc.named_scope(f"proj{tcb}"); scope.__enter__()
                    qps = [
                        p1q.tile([128, 512], F32, name=f"qps{tcb}_{h}", tag=f"qps{h}")
                        for h in range(N_QH)
                    ]
                    kps = p1k.tile([128, 512], F32, name=f"kps{tcb}", tag="kps")
                    vtps = p1k.tile([128, 512], F32, name=f"vtps{tcb}", tag="vtps")
                    xt_g = None
                    for kc in range(N_KC):
                        if kc % 8 == 0:
                            xt_g = xtp.tile([128, 8, 512], BF16, name=f"xt{tcb}_{kc}", tag="xt")
                            nc.sync.dma_start(
                                out=xt_g[:],
                                in_=xt_r[kc:kc + 8, :, t0:t0 + 512].rearrange("g p n -> p g n"),
                            )
                        xt_t = xt_g[:, kc % 8, :]
                        st, sp = kc == 0, kc == N_KC - 1
                        # v/k first: their PSUM banks are freed earliest by
                        # the rope/copy chain, so the next chunk's leading
                        # matmuls stall least on single-buffered banks.
                        nc.tensor.matmul(vtps[:], wv_sb[:, kc, :], xt_t, start=st, stop=sp)
                        nc.tensor.matmul(kps[:], wk_sb[:, kc, :], xt_t, start=st, stop=sp)
                        for h in range(N_QH):
                            nc.tensor.matmul(
                                qps[h][:], wq_sb[:, kc, h * 128:(h + 1) * 128],
                                xt_t, start=st, stop=sp,
                            )

                    # V chunk evacuation first (frees vtps), then rope.
                    vt_sb = vtp.tile([128, 512], BF16, name=f"vts{tcb}", tag="vts")
                    nc.vector.tensor_copy(vt_sb[:], vtps[:])

                    # rope on DVE: K first (attention depends on full KT).
                    # With deinterleaved chans, cs1=[cos;-sin], cs2=[sin;cos]:
                    #   a = [v1*c ; v1*s] (both from src_lo),
                    #   b = [-v2*s ; v2*c] (both from src_hi),  r = a + b.
                    # Every mul keeps PSUM-in base == SBUF-in base (legal);
                    # only the out base differs. The add is one full-lane op.
                    for h in [N_QH] + list(range(N_QH)):
                        src = kps if h == N_QH else qps[h]
                        a_t = uvp.tile([128, 512], BF16, name=f"u{tcb}_{h}", tag="u")
                        b_t = uvp.tile([128, 512], BF16, name=f"v{tcb}_{h}", tag="v")
                        nc.vector.tensor_mul(a_t[0:64, :], src[0:64, :], cs1_sb[0:64, t0:t0 + 512])
                        nc.vector.tensor_mul(a_t[64:128, :], src[0:64, :], cs2_sb[0:64, t0:t0 + 512])
                        nc.vector.tensor_mul(b_t[0:64, :], src[64:128, :], cs1_sb[64:128, t0:t0 + 512])
                        nc.vector.tensor_mul(b_t[64:128, :], src[64:128, :], cs2_sb[64:128, t0:t0 + 512])
                        if h == N_QH:
                            dst = ktt[tcb][:]
                        else:
                            dst = qtt[tcb][:, h, :]
                        nc.vector.tensor_add(dst, a_t[:], b_t[:])

                    # VT -> PE transpose -> V
                    vtr = p1r.tile([128, 4, 128], BF16, name=f"vtr{tcb}", tag="vtr")
                    for j in range(4):
                        nc.tensor.transpose(
                            vtr[:, j, :], vt_sb[:, j * 128:(j + 1) * 128],
                            ident_sb[:],
                        )
                    nc.vector.tensor_copy(v_sb[:, tcb * 4:(tcb + 1) * 4, :], vtr[:])
                    scope.__exit__(None, None, None)

            # -- phase 2: attention, with prev chunk's local wo GEMM woven in --
            with (
                tc.tile_pool(name="wo", bufs=1) as wop,
                tc.tile_pool(name="ep", bufs=6) as ep,
                tc.tile_pool(name="zp", bufs=1) as zp,
                tc.tile_pool(name="np_", bufs=2) as np_,
                tc.tile_pool(name="atp", bufs=2) as atp,
                tc.tile_pool(name="rsev", bufs=8) as rsev,
                tc.tile_pool(name="p2s", bufs=2, space="PSUM") as p2s,
                tc.tile_pool(name="p2pv", bufs=1, space="PSUM") as p2pv,
                tc.tile_pool(name="p2wo", bufs=2, space="PSUM") as p2wo,
            ):
                wo_sb = wop.tile([128, N_QH, D], BF16)
                nc.gpsimd.dma_start(out=wo_sb[:], in_=wot[:])

                at_t = {}      # (qc, h) -> normalized attention tile [128, 512]
                wo_wps = {}

                def fire_rs(ri):
                    # rs_in[ri] is complete: reduce-scatter it. The rank-
                    # shard -> output copy for half ri-2 is emitted right
                    # after this trigger: by then that RS has long finished,
                    # so the copy's wait never delays a later trigger.
                    # Everything lives on the gpsimd queue; the sync queue
                    # must stay free for rs_in writes or the weave wedges
                    # on evac-buffer recycling.
                    sc2 = nc.named_scope(f"rs{ri}"); sc2.__enter__()
                    nc.gpsimd.collective_compute(
                        "ReduceScatter",
                        mybir.AluOpType.add,
                        replica_groups=[list(range(N_CORES))],
                        ins=[rs_in[ri][:].opt()],
                        outs=[rs_out[ri][:].opt()],
                    )
                    sc2.__exit__(None, None, None)
                    if ri >= 2:
                        emit_out(ri - 2)
                    if ri == 7:
                        emit_out(6)
                        emit_out(7)

                def emit_out(ri):
                    nc.gpsimd.dma_start(
                        out=out_ext[ri * 256:(ri + 1) * 256, :], in_=rs_out[ri][:]
                    )

                def wo_mm(qc, m):
                    # one wo matmul: m = ts*32 + ob*4 + ct
                    # out_partial[ts-tile, ob*512:+512] += at[ct][:,ts].T @ woT[ct, ob]
                    ts, ob, ct = m // 32, (m % 32) // 4, m % 4
                    if ct == 0:
                        wo_wps[qc] = p2wo.tile(
                            [128, 512], F32, name=f"wps{qc}_{m}", tag="wo"
                        )
                    wps = wo_wps[qc]
                    nc.tensor.matmul(
                        wps[:],
                        at_t[(qc, ct)][:, ts * 128:(ts + 1) * 128],
                        wo_sb[:, ct, ob * 512:(ob + 1) * 512],
                        start=(ct == 0), stop=(ct == 3),
                    )
                    if ct == 3:
                        ev = rsev.tile(
                            [128, 512], BF16, name=f"wev{qc}_{m}", tag="wev"
                        )
                        nc.vector.tensor_copy(ev[:], wps[:])
                        ri, row = qc * 2 + ts // 2, (ts % 2) * 128
                        nc.sync.dma_start(
                            out=rs_in[ri][ob, row:row + 128, :], in_=ev[:]
                        )
                    if m == 63:
                        fire_rs(qc * 2)
                    elif m == 127:
                        fire_rs(qc * 2 + 1)

                def attn_chunk(qc, weave=None):
                    # weave: chunk index whose wo matmuls (128) are spread
                    # over this chunk's kt slots, 8 per slot (done by mid-
                    # chunk so the RS halves launch early). The wo matmuls
                    # sit BETWEEN this slot's scores and PV so the PV's
                    # wait on the exp stream is absorbed by wo work.
                    scope = nc.named_scope(f"attn{qc}"); scope.__enter__()
                    wv_n = [0]

                    def weave_step(upto):
                        if weave is None:
                            return
                        while wv_n[0] < min(upto, 128):
                            wo_mm(weave, wv_n[0])
                            wv_n[0] += 1

                    for hp in range(2):
                        hs = [2 * hp, 2 * hp + 1]
                        pvs = {
                            h: p2pv.tile([128, 512], F32, name=f"pv{qc}_{h}", tag=f"pv{h % 2}")
                            for h in hs
                        }
                        zparts = {
                            h: zp.tile([128, 512], BF16, name=f"zpt{qc}_{h}", tag=f"zpart{h % 2}")
                            for h in hs
                        }
                        for kt in range(N_KT):
                            kt_st = ktt[kt // 4][:, (kt % 4) * 128:(kt % 4 + 1) * 128]
                            stp = p2s.tile([128, 2, 512], F32, name=f"st{qc}_{hp}_{kt}", tag="st")
                            for j, h in enumerate(hs):
                                nc.tensor.matmul(
                                    stp[:, j, :], kt_st,
                                    qtt[qc][:, h, :], start=True, stop=True,
                                )
                            e_t = ep.tile([128, 2, 512], BF16, name=f"e{qc}_{kt}_{hp}", tag="e")
                            nc.scalar.activation(
                                out=e_t[:], in_=stp[:],
                                func=mybir.ActivationFunctionType.Exp,
                            )
                            for j, h in enumerate(hs):
                                if kt == 0:
                                    nc.vector.tensor_copy(zparts[h][:], e_t[:, j, :])
                                else:
                                    nc.vector.tensor_add(zparts[h][:], zparts[h][:], e_t[:, j, :])
                            weave_step((hp * N_KT + kt + 1) * 8)
                            for j, h in enumerate(hs):
                                nc.tensor.matmul(
                                    pvs[h][:], v_sb[:, kt, :], e_t[:, j, :],
                                    start=(kt == 0), stop=(kt == N_KT - 1),
                                )
                        zpss = {}
                        for h in hs:
                            zpss[h] = p2s.tile([1, 512], F32, name=f"zps{qc}_{h}", tag="st")
                            nc.tensor.matmul(zpss[h][:], onesc_sb[:], zparts[h][:], start=True, stop=True)
                        invzs = {}
                        for h in hs:
                            lnz = zp.tile([1, 512], F32, name=f"lnz{qc}_{h}", tag=f"lnz{h % 2}")
                            nc.scalar.activation(
                                out=lnz[:], in_=zpss[h][:],
                                func=mybir.ActivationFunctionType.Ln,
                            )
                            invzs[h] = zp.tile([1, 512], F32R, name=f"izr{qc}_{h}", tag=f"invz{h % 2}")
                            nc.scalar.activation(
                                out=invzs[h][:], in_=lnz[:],
                                func=mybir.ActivationFunctionType.Exp, scale=-1.0,
                            )
                        for h in hs:
                            bcps = p2s.tile([128, 512], F32, name=f"bc{qc}_{h}", tag="st")
                            nc.tensor.matmul(bcps[:], onesr_sb[:], invzs[h][:], start=True, stop=True)
                            bc_sb = np_.tile([128, 512], F32, name=f"bcs{qc}_{h}", tag="bcs")
                            nc.vector.tensor_copy(bc_sb[:], bcps[:])
                            a_t = atp.tile([128, 512], BF16, name=f"at{qc}_{h}", tag=f"at{h}")
                            nc.vector.tensor_mul(a_t[:], pvs[h][:], bc_sb[:])
                            at_t[(qc, h)] = a_t
                    weave_step(128)
                    scope.__exit__(None, None, None)

                attn_chunk(0)
                attn_chunk(1, weave=0)
                attn_chunk(2, weave=1)
                attn_chunk(3, weave=2)
                scope = nc.named_scope("wo3"); scope.__enter__()
                for m in range(128):
                    wo_mm(3, m)
                scope.__exit__(None, None, None)

    _split_multi_waits(nc)
    return nc


def _host_prep(x, cos, sin, wq, wk, wv, wo):
    scale = np.float32(HD ** -0.5)
    perm = np.concatenate([np.arange(0, HD, 2), np.arange(1, HD, 2)])

    xT = np.ascontiguousarray(x.T.astype(_BF16))
    cosT = np.ascontiguousarray(cos.T)
    sinT = np.ascontiguousarray(sin.T)
    cs1 = np.concatenate([cosT, -sinT], axis=0)
    cs2 = np.concatenate([sinT, cosT], axis=0)

    shared = {
        "xt": xT,
        "cs1": cs1,
        "cs2": cs2,
        "onesc": np.ones((HD, 1), np.float32).astype(_BF16),
        "onesr": np.ones((1, HD), np.float32),
        "ident": np.eye(HD, dtype=np.float32).astype(_BF16),
    }
    in_maps = []
    for c in range(N_CORES):
        wq_c = wq[c * 512:(c + 1) * 512].reshape(N_QH, HD, D)[:, perm, :]
        wq_c = (wq_c.reshape(512, D) * scale)
        wk_c = wk[c * HD:(c + 1) * HD][perm, :]
        wv_c = wv[c * HD:(c + 1) * HD]
        # woT for the local-wo GEMM: [p, ct, n] with chan = c*512 + ct*128 + p
        wo_c = np.ascontiguousarray(
            wo[:, c * 512:(c + 1) * 512].T.reshape(N_QH, 128, D).transpose(1, 0, 2)
        )
        m = dict(shared)
        m["wqt"] = np.ascontiguousarray(wq_c.T).astype(_BF16)
        m["wkt"] = np.ascontiguousarray(wk_c.T).astype(_BF16)
        m["wvt"] = np.ascontiguousarray(wv_c.T).astype(_BF16)
        m["wot"] = wo_c.astype(_BF16)
        in_maps.append(m)
    return in_maps


def kernel(x, cos, sin, wq, wk, wv, wo, _trace=False):
    x = np.asarray(x, np.float32)
    cos = np.asarray(cos, np.float32)
    sin = np.asarray(sin, np.float32)
    wq = np.asarray(wq, np.float32)
    wk = np.asarray(wk, np.float32)
    wv = np.asarray(wv, np.float32)
    wo = np.asarray(wo, np.float32)

    in_maps = _host_prep(x, cos, sin, wq, wk, wv, wo)
    if "nc" not in _NC_CACHE:
        _NC_CACHE["nc"] = _build()
    nc = _NC_CACHE["nc"]
    res = run_bass_kernel_spmd(
        nc, in_maps, core_ids=list(range(N_CORES)), trace=_trace
    )
    out = np.concatenate(
        [np.asarray(res.results[c]["out"]) for c in range(N_CORES)], axis=1
    )
    out = np.ascontiguousarray(out.astype(np.float32))
    if _trace:
        kernel._last_exec_time_ns = res.exec_time_ns
        kernel._last_result = res
    return out


# revision 32
# speedup vs baseline: 1.3652x; 1.1034x over previous
"""GQA attention (S=2048, D=4096, 32 Q heads / 8 KV heads, RoPE, full attn)
distributed over 8 Trainium2 NeuronCores.

Strategy (tensor-parallel by heads, local-wo + ReduceScatter):
  - core c owns Q heads 4c..4c+3 and KV head c (GQA groups align with cores).
  - projections as transposed GEMMs: QT/KT [chan, tok] directly usable by
    the scores matmul; V via VT + PE transposes; all big matmuls bf16
    (Fast Weight Load), f32 PSUM accumulate.
  - RoPE on the DVE only: with host-deinterleaved channels, r1 = u_lo - u_hi
    and r2 = v_lo + v_hi where u = src*cs1, v = src*cs2 (partition-split
    operands; PSUM src + SBUF table makes the mixed-base read legal).
  - scores transposed, ST = KT.T @ QT -> [k, q]; exp on ScalarE (bf16 out);
    softmax normalizer: bf16 DVE partial sums -> ones-matmul -> ln/exp on
    ScalarE -> f32r broadcast-matmul -> DVE scale.
  - output projection is LOCAL: out_partial[tok, 4096] = at_local.T @ woT
    (stationary = attention tile, moving = woT blocks), quantized bf16 and
    ReduceScattered per q-chunk; rank c's shard is exactly its out columns.
    No AllGather, no gather staging, and wo never waits on a collective.
  - wo(qc) matmuls are WOVEN into attn(qc+1)'s kt loop (4 per slot): the PE
    queue never idles while the scalar engine streams exps.
  - a full-sized dummy ReduceScatter during the projections absorbs the
    ncfw cold-start + RDH algorithm first-use cost (~60us -> ~25us).
  - consecutive matmuls that reuse the same bf16 stationary operand get
    ldweights=False patched post-schedule (skips the ~50ns serialized
    weight load; scores/PV pairs, wo jo-pairs, zps pairs).

Host side only reshapes/transposes/pads/casts and concatenates outputs
(final bf16 -> f32 upcast included).
"""
import sys

import numpy as np
import ml_dtypes

_BF16 = ml_dtypes.bfloat16

for _p in ("/root/.axon_site/_ro/trn_rl_repo", "/opt/trn_rl_repo"):
    if _p not in sys.path:
        sys.path.append(_p)

import concourse.bass as bass
import concourse.tile as tile
from concourse import mybir
from concourse.bass_utils import run_bass_kernel_spmd

N_CORES = 8
S = 2048
D = 4096
HD = 128
N_QH = 4          # Q heads per core
N_KT = S // 128   # 16 k-tiles
N_TC = S // 512   # 4 token chunks
N_KC = D // 128   # 32 contraction tiles
F32 = mybir.dt.float32
F32R = mybir.dt.float32r
BF16 = mybir.dt.bfloat16

_NC_CACHE = {}


def _split_multi_waits(nc):
    """This container's walrus accepts only ONE sync-wait per instruction
    encoding; hoist extra waits onto fresh single-wait NoOps placed before
    the instruction on the same engine."""
    n = 0
    for fn in nc.m.functions:
        for bb in fn.blocks:
            new_insts = []
            changed = False
            for ins in bb.instructions:
                si = ins.sync_info
                waits = list(si.on_wait) if si is not None else []
                if len(waits) > 1:
                    for w in waits[:-1]:
                        n += 1
                        nop = mybir.InstNoOp(name=f"WSPL-{n}", ins=[], outs=[])
                        nop.engine = ins.engine
                        nop.sync_info = mybir.SyncInfo(on_wait=[w], on_update=[])
                        new_insts.append(nop)
                    si.on_wait = waits[-1:]
                    changed = True
                new_insts.append(ins)
            if changed:
                bb.instructions = new_insts
    return n


def _dedup_ldweights(nc):
    """Consecutive matmuls (no other tensor-engine instruction between)
    with an identical bf16 stationary operand: the later ones reuse the
    already-loaded PE array weights (ldweights=False). f32/f32r stationary
    is excluded (non-self-loading f32r matmuls return zeros on HW)."""
    n = 0
    for fn in nc.m.functions:
        for bb in fn.blocks:
            prev = None  # (engine, weights_repr) of last tensor-engine inst
            for ins in bb.instructions:
                if isinstance(ins, mybir.InstMatmult):
                    if ins.is_transpose:
                        prev = None
                        continue
                    w = ins.ins[1]
                    wrep = str(w)
                    ok_dtype = "bfloat16" in wrep
                    if (
                        prev is not None
                        and wrep == prev
                        and ok_dtype
                        and ins.perf_mode is None
                    ):
                        ins.ldweights = False
                        n += 1
                    prev = wrep
                elif isinstance(ins, mybir.InstLdweights):
                    prev = None
                # other engines' instructions don't touch PE weights
    return n


def _build():
    nc = bass.Bass()

    xt = nc.dram_tensor("xt", [D, S], BF16, kind="ExternalInput")
    wqt = nc.dram_tensor("wqt", [D, 512], BF16, kind="ExternalInput")
    wkt = nc.dram_tensor("wkt", [D, HD], BF16, kind="ExternalInput")
    wvt = nc.dram_tensor("wvt", [D, HD], BF16, kind="ExternalInput")
    wot = nc.dram_tensor("wot", [128, N_QH, D], BF16, kind="ExternalInput")
    cs1 = nc.dram_tensor("cs1", [HD, S], BF16, kind="ExternalInput")
    cs2 = nc.dram_tensor("cs2", [HD, S], BF16, kind="ExternalInput")
    onesc = nc.dram_tensor("onesc", [HD, 1], BF16, kind="ExternalInput")
    onesr = nc.dram_tensor("onesr", [1, HD], F32R, kind="ExternalInput")
    ident = nc.dram_tensor("ident", [HD, HD], BF16, kind="ExternalInput")
    out_ext = nc.dram_tensor("out", [S, 512], BF16, kind="ExternalOutput")

    # ReduceScatter buffers, one per 256-token half-chunk: rs_in[b, t, n]
    # holds the partial contribution to out columns b*512..(b+1)*512 ->
    # rank b receives the b-th contiguous shard of the sum = its own out
    # columns. Half-chunks keep each CC op ~26us so the tail pair drains
    # fast and the stream never backs up.
    rs_in = [
        nc.dram_tensor(f"rsi{i}", [N_CORES, 256, 512], BF16) for i in range(8)
    ]
    rs_out = [
        nc.dram_tensor(f"rso{i}", [256, 512], BF16) for i in range(8)
    ]
    # same-shape dummy RS fired during the projections: absorbs the ncfw
    # cold-start AND the algorithm first-use cost so RS0 runs hot.
    warm_in = nc.dram_tensor("warmi", [N_CORES, 256, 512], BF16)
    warm_out = nc.dram_tensor("warmo", [256, 512], BF16)

    xt_r = xt.rearrange("(kc p) s -> kc p s", p=128)
    wqt_r = wqt.rearrange("(kc p) n -> kc p n", p=128)
    wkt_r = wkt.rearrange("(kc p) n -> kc p n", p=128)
    wvt_r = wvt.rearrange("(kc p) n -> kc p n", p=128)


    with tile.TileContext(nc) as tc:
        with (
            tc.tile_pool(name="const", bufs=1) as constp,
            tc.tile_pool(name="persist", bufs=1) as persist,
        ):
            onesc_sb = constp.tile([HD, 1], BF16)
            onesr_sb = constp.tile([1, HD], F32R)

            # per-token-chunk tiles (NOT one big tile): Tile's dependency
            # tracking is per-tile, so attn chunk 0's first scores must not
            # falsely wait on proj3's rope writes.
            qtt = [
                persist.tile([128, N_QH, 512], BF16, name=f"qtt{i}")
                for i in range(N_TC)
            ]
            ktt = [
                persist.tile([128, 512], BF16, name=f"ktt{i}")
                for i in range(N_TC)
            ]
            v_sb = persist.tile([128, N_KT, HD], BF16)   # V [tok-in-tile, kt, chan]

            # ---------------- phase 1: projections + rope ----------------
            with (
                tc.tile_pool(name="wq", bufs=1) as wqp,
                tc.tile_pool(name="csp", bufs=1) as csp,
                tc.tile_pool(name="xtp", bufs=3) as xtp,
                tc.tile_pool(name="uv", bufs=2) as uvp,
                tc.tile_pool(name="vt", bufs=2) as vtp,
                tc.tile_pool(name="p1q", bufs=1, space="PSUM") as p1q,
                tc.tile_pool(name="p1k", bufs=1, space="PSUM") as p1k,
                tc.tile_pool(name="p1r", bufs=1, space="PSUM") as p1r,
            ):
                wq_sb = wqp.tile([128, N_KC, 512], BF16)
                wk_sb = wqp.tile([128, N_KC, HD], BF16)
                wv_sb = wqp.tile([128, N_KC, HD], BF16)
                cs1_sb = csp.tile([HD, S], BF16)
                cs2_sb = csp.tile([HD, S], BF16)
                ident_sb = csp.tile([HD, HD], BF16)

                nc.gpsimd.collective_compute(
                    "ReduceScatter",
                    mybir.AluOpType.add,
                    replica_groups=[list(range(N_CORES))],
                    ins=[warm_in[:].opt()],
                    outs=[warm_out[:].opt()],
                )
                # weights in 8-kc groups on gpsimd, interleaved wq/wk/wv in
                # consumption order (xt rides the sync queue in parallel);
                # cs tables before the last group (rope needs them ~55us in).
                for ch in range(4):
                    if ch == 3:
                        nc.gpsimd.dma_start(out=cs1_sb[:], in_=cs1[:])
                        nc.gpsimd.dma_start(out=cs2_sb[:], in_=cs2[:])
                    nc.gpsimd.dma_start(
                        out=wq_sb[:, ch * 8:(ch + 1) * 8, :],
                        in_=wqt_r[ch * 8:(ch + 1) * 8].rearrange("kc p n -> p kc n"),
                    )
                    nc.gpsimd.dma_start(
                        out=wk_sb[:, ch * 8:(ch + 1) * 8, :],
                        in_=wkt_r[ch * 8:(ch + 1) * 8].rearrange("kc p n -> p kc n"),
                    )
                    nc.gpsimd.dma_start(
                        out=wv_sb[:, ch * 8:(ch + 1) * 8, :],
                        in_=wvt_r[ch * 8:(ch + 1) * 8].rearrange("kc p n -> p kc n"),
                    )
                nc.gpsimd.dma_start(out=onesc_sb[:], in_=onesc[:])
                nc.gpsimd.dma_start(out=onesr_sb[:], in_=onesr[:])
                nc.gpsimd.dma_start(out=ident_sb[:], in_=ident[:])

                for tcb in range(N_TC):
                    t0 = tcb * 512
                    scope = nc.named_scope(f"proj{tcb}"); scope.__enter__()
                    qps = [
                        p1q.tile([128, 512], F32, name=f"qps{tcb}_{h}", tag=f"qps{h}")
                        for h in range(N_QH)
                    ]
                    kps = p1k.tile([128, 512], F32, name=f"kps{tcb}", tag="kps")
                    vtps = p1k.tile([128, 512], F32, name=f"vtps{tcb}", tag="vtps")
                    xt_g = None
                    for kc in range(N_KC):
                        if kc % 8 == 0:
                            xt_g = xtp.tile([128, 8, 512], BF16, name=f"xt{tcb}_{kc}", tag="xt")
                            nc.sync.dma_start(
                                out=xt_g[:],
                                in_=xt_r[kc:kc + 8, :, t0:t0 + 512].rearrange("g p n -> p g n"),
                            )
                        xt_t = xt_g[:, kc % 8, :]
                        st, sp = kc == 0, kc == N_KC - 1
                        # v/k first: their PSUM banks are freed earliest by
                        # the rope/copy chain, so the next chunk's leading
                        # matmuls stall least on single-buffered banks.
                        nc.tensor.matmul(vtps[:], wv_sb[:, kc, :], xt_t, start=st, stop=sp)
                        nc.tensor.matmul(kps[:], wk_sb[:, kc, :], xt_t, start=st, stop=sp)
                        for h in range(N_QH):
                            nc.tensor.matmul(
                                qps[h][:], wq_sb[:, kc, h * 128:(h + 1) * 128],
                                xt_t, start=st, stop=sp,
                            )

                    # V chunk evacuation first (frees vtps), then the k/q
                    # PSUM banks are drained by SCALAR copies (ScalarE reads
                    # PSUM; it is idle all through phase 1) so the next
                    # chunk's matmuls and phase 2's PSUM reuse never wait on
                    # the serial DVE rope chain.
                    vt_sb = vtp.tile([128, 512], BF16, name=f"vts{tcb}", tag="vts")
                    nc.vector.tensor_copy(vt_sb[:], vtps[:])
                    pcp = {}
                    for h in [N_QH] + list(range(N_QH)):
                        src = kps if h == N_QH else qps[h]
                        cp = uvp.tile([128, 512], BF16, name=f"pc{tcb}_{h}", tag=f"pc{h}")
                        nc.scalar.activation(
                            out=cp[:], in_=src[:],
                            func=mybir.ActivationFunctionType.Copy,
                        )
                        pcp[h] = cp

                    # rope on DVE: K first (attention depends on full KT).
                    # With deinterleaved chans, cs1=[cos;-sin], cs2=[sin;cos]:
                    #   a = [v1*c ; v1*s] (both from src_lo),
                    #   b = [-v2*s ; v2*c] (both from src_hi),  r = a + b.
                    # All-bf16 SBUF operands -> 2x DVE rate; the partition-
                    # base mismatch needs out-base==0... (in-bases equal per
                    # mul; out base may differ).
                    for h in [N_QH] + list(range(N_QH)):
                        src = pcp[h]
                        a_t = uvp.tile([128, 512], BF16, name=f"u{tcb}_{h}", tag="u")
                        b_t = uvp.tile([128, 512], BF16, name=f"v{tcb}_{h}", tag="v")
                        nc.vector.tensor_mul(a_t[0:64, :], src[0:64, :], cs1_sb[0:64, t0:t0 + 512])
                        nc.vector.tensor_mul(a_t[64:128, :], src[0:64, :], cs2_sb[0:64, t0:t0 + 512])
                        nc.vector.tensor_mul(b_t[0:64, :], src[64:128, :], cs1_sb[64:128, t0:t0 + 512])
                        nc.vector.tensor_mul(b_t[64:128, :], src[64:128, :], cs2_sb[64:128, t0:t0 + 512])
                        if h == N_QH:
                            dst = ktt[tcb][:]
                        else:
                            dst = qtt[tcb][:, h, :]
                        nc.vector.tensor_add(dst, a_t[:], b_t[:])

                    # VT -> PE transpose -> V
                    vtr = p1r.tile([128, 4, 128], BF16, name=f"vtr{tcb}", tag="vtr")
                    for j in range(4):
                        nc.tensor.transpose(
                            vtr[:, j, :], vt_sb[:, j * 128:(j + 1) * 128],
                            ident_sb[:],
                        )
                    nc.vector.tensor_copy(v_sb[:, tcb * 4:(tcb + 1) * 4, :], vtr[:])
                    scope.__exit__(None, None, None)

            # -- phase 2: attention, with prev chunk's local wo GEMM woven in --
            with (
                tc.tile_pool(name="wo", bufs=1) as wop,
                tc.tile_pool(name="ep", bufs=6) as ep,
                tc.tile_pool(name="zp", bufs=1) as zp,
                tc.tile_pool(name="np_", bufs=2) as np_,
                tc.tile_pool(name="atp", bufs=2) as atp,
                tc.tile_pool(name="rsev", bufs=8) as rsev,
                tc.tile_pool(name="p2s", bufs=2, space="PSUM") as p2s,
                tc.tile_pool(name="p2pv", bufs=1, space="PSUM") as p2pv,
                tc.tile_pool(name="p2wo", bufs=2, space="PSUM") as p2wo,
            ):
                wo_sb = wop.tile([128, N_QH, D], BF16)
                nc.gpsimd.dma_start(out=wo_sb[:], in_=wot[:])

                at_t = {}      # (qc, h) -> normalized attention tile [128, 512]
                wo_wps = {}

                def fire_rs(ri):
                    # rs_in[ri] is complete: reduce-scatter it. The rank-
                    # shard -> output copy for half ri-2 is emitted right
                    # after this trigger: by then that RS has long finished,
                    # so the copy's wait never delays a later trigger.
                    # Everything lives on the gpsimd queue; the sync queue
                    # must stay free for rs_in writes or the weave wedges
                    # on evac-buffer recycling.
                    sc2 = nc.named_scope(f"rs{ri}"); sc2.__enter__()
                    nc.gpsimd.collective_compute(
                        "ReduceScatter",
                        mybir.AluOpType.add,
                        replica_groups=[list(range(N_CORES))],
                        ins=[rs_in[ri][:].opt()],
                        outs=[rs_out[ri][:].opt()],
                    )
                    sc2.__exit__(None, None, None)
                    if ri >= 2:
                        emit_out(ri - 2)
                    if ri == 7:
                        emit_out(6)
                        emit_out(7)

                def emit_out(ri):
                    nc.gpsimd.dma_start(
                        out=out_ext[ri * 256:(ri + 1) * 256, :], in_=rs_out[ri][:]
                    )

                def wo_mm(qc, m):
                    # one wo matmul: m = ts*32 + ob*4 + ct
                    # out_partial[ts-tile, ob*512:+512] += at[ct][:,ts].T @ woT[ct, ob]
                    ts, ob, ct = m // 32, (m % 32) // 4, m % 4
                    if ct == 0:
                        wo_wps[qc] = p2wo.tile(
                            [128, 512], F32, name=f"wps{qc}_{m}", tag="wo"
                        )
                    wps = wo_wps[qc]
                    nc.tensor.matmul(
                        wps[:],
                        at_t[(qc, ct)][:, ts * 128:(ts + 1) * 128],
                        wo_sb[:, ct, ob * 512:(ob + 1) * 512],
                        start=(ct == 0), stop=(ct == 3),
                    )
                    if ct == 3:
                        ev = rsev.tile(
                            [128, 512], BF16, name=f"wev{qc}_{m}", tag="wev"
                        )
                        nc.vector.tensor_copy(ev[:], wps[:])
                        ri, row = qc * 2 + ts // 2, (ts % 2) * 128
                        nc.sync.dma_start(
                            out=rs_in[ri][ob, row:row + 128, :], in_=ev[:]
                        )
                    if m == 63:
                        fire_rs(qc * 2)
                    elif m == 127:
                        fire_rs(qc * 2 + 1)

                def attn_chunk(qc, weave=None):
                    # weave: chunk index whose wo matmuls (128) are spread
                    # over this chunk's kt slots, 8 per slot (done by mid-
                    # chunk so the RS halves launch early). The wo matmuls
                    # sit BETWEEN this slot's scores and PV so the PV's
                    # wait on the exp stream is absorbed by wo work.
                    scope = nc.named_scope(f"attn{qc}"); scope.__enter__()
                    wv_n = [0]

                    def weave_step(upto):
                        if weave is None:
                            return
                        while wv_n[0] < min(upto, 128):
                            wo_mm(weave, wv_n[0])
                            wv_n[0] += 1

                    for hp in range(2):
                        hs = [2 * hp, 2 * hp + 1]
                        pvs = {
                            h: p2pv.tile([128, 512], F32, name=f"pv{qc}_{h}", tag=f"pv{h % 2}")
                            for h in hs
                        }
                        zparts = {
                            h: zp.tile([128, 512], BF16, name=f"zpt{qc}_{h}", tag=f"zpart{h % 2}")
                            for h in hs
                        }
                        for kt in range(N_KT):
                            kt_st = ktt[kt // 4][:, (kt % 4) * 128:(kt % 4 + 1) * 128]
                            stp = p2s.tile([128, 2, 512], F32, name=f"st{qc}_{hp}_{kt}", tag="st")
                            for j, h in enumerate(hs):
                                nc.tensor.matmul(
                                    stp[:, j, :], kt_st,
                                    qtt[qc][:, h, :], start=True, stop=True,
                                )
                            e_t = ep.tile([128, 2, 512], BF16, name=f"e{qc}_{kt}_{hp}", tag="e")
                            nc.scalar.activation(
                                out=e_t[:], in_=stp[:],
                                func=mybir.ActivationFunctionType.Exp,
                            )
                            for j, h in enumerate(hs):
                                if kt == 0:
                                    nc.vector.tensor_copy(zparts[h][:], e_t[:, j, :])
                                else:
                                    nc.vector.tensor_add(zparts[h][:], zparts[h][:], e_t[:, j, :])
                            weave_step((hp * N_KT + kt + 1) * 4)
                            for j, h in enumerate(hs):
                                nc.tensor.matmul(
                                    pvs[h][:], v_sb[:, kt, :], e_t[:, j, :],
                                    start=(kt == 0), stop=(kt == N_KT - 1),
                                )
                        zpss = {}
                        for h in hs:
                            zpss[h] = p2s.tile([1, 512], F32, name=f"zps{qc}_{h}", tag="st")
                            nc.tensor.matmul(zpss[h][:], onesc_sb[:], zparts[h][:], start=True, stop=True)
                        invzs = {}
                        for h in hs:
                            lnz = zp.tile([1, 512], F32, name=f"lnz{qc}_{h}", tag=f"lnz{h % 2}")
                            nc.scalar.activation(
                                out=lnz[:], in_=zpss[h][:],
                                func=mybir.ActivationFunctionType.Ln,
                            )
                            invzs[h] = zp.tile([1, 512], F32R, name=f"izr{qc}_{h}", tag=f"invz{h % 2}")
                            nc.scalar.activation(
                                out=invzs[h][:], in_=lnz[:],
                                func=mybir.ActivationFunctionType.Exp, scale=-1.0,
                            )
                        for h in hs:
                            bcps = p2s.tile([128, 512], F32, name=f"bc{qc}_{h}", tag="st")
                            nc.tensor.matmul(bcps[:], onesr_sb[:], invzs[h][:], start=True, stop=True)
                            bc_sb = np_.tile([128, 512], F32, name=f"bcs{qc}_{h}", tag="bcs")
                            nc.vector.tensor_copy(bc_sb[:], bcps[:])
                            a_t = atp.tile([128, 512], BF16, name=f"at{qc}_{h}", tag=f"at{h}")
                            nc.vector.tensor_mul(a_t[:], pvs[h][:], bc_sb[:])
                            at_t[(qc, h)] = a_t
                    weave_step(128)
                    scope.__exit__(None, None, None)

                attn_chunk(0)
                attn_chunk(1, weave=0)
                attn_chunk(2, weave=1)
                attn_chunk(3, weave=2)
                scope = nc.named_scope("wo3"); scope.__enter__()
                for m in range(128):
                    wo_mm(3, m)
                scope.__exit__(None, None, None)

    _split_multi_waits(nc)
    return nc


def _host_prep(x, cos, sin, wq, wk, wv, wo):
    scale = np.float32(HD ** -0.5)
    perm = np.concatenate([np.arange(0, HD, 2), np.arange(1, HD, 2)])

    xT = np.ascontiguousarray(x.T.astype(_BF16))
    cosT = np.ascontiguousarray(cos.T)
    sinT = np.ascontiguousarray(sin.T)
    cs1 = np.concatenate([cosT, -sinT], axis=0).astype(_BF16)
    cs2 = np.concatenate([sinT, cosT], axis=0).astype(_BF16)

    shared = {
        "xt": xT,
        "cs1": cs1,
        "cs2": cs2,
        "onesc": np.ones((HD, 1), np.float32).astype(_BF16),
        "onesr": np.ones((1, HD), np.float32),
        "ident": np.eye(HD, dtype=np.float32).astype(_BF16),
    }
    in_maps = []
    for c in range(N_CORES):
        wq_c = wq[c * 512:(c + 1) * 512].reshape(N_QH, HD, D)[:, perm, :]
        wq_c = (wq_c.reshape(512, D) * scale)
        wk_c = wk[c * HD:(c + 1) * HD][perm, :]
        wv_c = wv[c * HD:(c + 1) * HD]
        # woT for the local-wo GEMM: [p, ct, n] with chan = c*512 + ct*128 + p
        wo_c = np.ascontiguousarray(
            wo[:, c * 512:(c + 1) * 512].T.reshape(N_QH, 128, D).transpose(1, 0, 2)
        )
        m = dict(shared)
        m["wqt"] = np.ascontiguousarray(wq_c.T).astype(_BF16)
        m["wkt"] = np.ascontiguousarray(wk_c.T).astype(_BF16)
        m["wvt"] = np.ascontiguousarray(wv_c.T).astype(_BF16)
        m["wot"] = wo_c.astype(_BF16)
        in_maps.append(m)
    return in_maps


def kernel(x, cos, sin, wq, wk, wv, wo, _trace=False):
    x = np.asarray(x, np.float32)
    cos = np.asarray(cos, np.float32)
    sin = np.asarray(sin, np.float32)
    wq = np.asarray(wq, np.float32)
    wk = np.asarray(wk, np.float32)
    wv = np.asarray(wv, np.float32)
    wo = np.asarray(wo, np.float32)

    in_maps = _host_prep(x, cos, sin, wq, wk, wv, wo)
    if "nc" not in _NC_CACHE:
        _NC_CACHE["nc"] = _build()
    nc = _NC_CACHE["nc"]
    res = run_bass_kernel_spmd(
        nc, in_maps, core_ids=list(range(N_CORES)), trace=_trace
    )
    out = np.concatenate(
        [np.asarray(res.results[c]["out"]) for c in range(N_CORES)], axis=1
    )
    out = np.ascontiguousarray(out.astype(np.float32))
    if _trace:
        kernel._last_exec_time_ns = res.exec_time_ns
        kernel._last_result = res
    return out
